# revision 1
# baseline (speedup 1.0000x reference)
"""GCN (2-layer, symmetric-normalized A+I) on 8 Trainium2 NeuronCores.

Strategy (node-range sharded, one AllGather):
  - deg/dinv computed on host from edge_index (int preprocessing / "degrees").
  - Core k owns dst rows [k*R, (k+1)*R).  All per-edge scaling is folded into
    per-edge gather weights so the device does: gather -> scale&cast ->
    one-hot matmul segment-sum in PSUM -> dense W1/relu/W2 -> AllGather of
    dinv-scaled hidden h2s -> same aggregation machinery -> log_softmax.
  - Aggregation: nodes are bin-packed into "windows" of <=128 nodes whose
    edges occupy 4 fixed-capacity chunk cells (gather src locality for int16
    indices) + 1 self cell.  Segment-sum = sum over 128-edge columns of
    S_col^T-style one-hot matmuls accumulated in PSUM.  The one-hot S is
    built on-device from per-edge dst-rank data via a single is_equal op.
  - All structure (window/cell/column/piece layout) is static and identical
    across cores; per-core variation lives only in input data arrays.

kernel(**inputs) takes FULL inputs and returns the FULL [100000, 64] output.
"""
import os
import numpy as np

os.environ.setdefault("NEURON_SCRATCHPAD_PAGE_SIZE", "256")

import concourse.bacc as bacc
import concourse.tile as tile
import concourse.mybir as mybir
from concourse import bass
from concourse.bass_utils import run_bass_kernel_spmd
from concourse.masks import make_identity

F32 = mybir.dt.float32
BF16 = mybir.dt.bfloat16
I16 = mybir.dt.int16
NPBF16 = mybir.dt.np(BF16)

AF = mybir.ActivationFunctionType
OP = mybir.AluOpType


class Cfg:
    def __init__(self, N, R, F, HID, C, NC, NWIN, NB, K, dense_bf16=False):
        self.N, self.R, self.F, self.HID, self.C, self.NC = N, R, F, HID, C, NC
        self.NWIN, self.NB, self.K = NWIN, NB, K
        self.WN = 128
        assert NWIN % NB == 0
        self.NBATCH = NWIN // NB
        assert (NB * K) % 128 == 0 and K % 16 == 0 and K >= 128
        # HW limit: DMAGatherAnt crashes for num_idxs > 1024 (empirical)
        assert NB * K <= 1024 and NB * 128 <= 1024
        self.NCOL = NB * K // 128          # columns per real call
        self.IDX_REAL = NB * K             # idxs per real call
        self.IDX_SELF = NB * 128           # idxs per self call
        assert N % 4 == 0
        self.CH1 = N // 4                  # L1 chunk rows
        self.CROWS = NWIN * 128            # compact rows per core
        self.GC = NC * self.CROWS          # global compact rows
        assert self.GC % 4 == 0
        self.CH2 = self.GC // 4            # L2 chunk rows
        assert self.CH1 <= 32768 and self.CH2 <= 32768 and self.CROWS <= 32768
        self.dense_bf16 = dense_bf16
        # static column->cell map for real calls: (first cell, crosses boundary)
        self.colmap = []
        for j in range(self.NCOL):
            a = (j * 128) // K
            self.colmap.append((a, (j * 128 + 127) // K != a))
        # stream free-dim widths (per batch)
        self.GIDX_B = 4 * self.IDX_REAL // 16 + self.IDX_SELF // 16
        self.DRA_B = 4 * self.NCOL + NB
        self.DRB_B = 4 * self.NCOL


CFG_FULL = Cfg(N=100000, R=12500, F=128, HID=256, C=64, NC=8,
               NWIN=100, NB=4, K=256)


# ----------------------------------------------------------------------------
# host prep
# ----------------------------------------------------------------------------

def _pack(cnt, cfg):
    """Bin-pack nodes (rows of cnt: per-chunk edge counts) into NWIN windows.
    Returns (assign, rank) or None."""
    NWIN, K, WN = cfg.NWIN, cfg.K, cfg.WN
    degs = cnt.sum(1)
    order = np.argsort(-degs, kind="stable")
    loads = np.zeros((NWIN, 4), np.int64)
    counts = np.zeros(NWIN, np.int64)
    assign = np.full(len(degs), -1, np.int64)
    rank = np.full(len(degs), -1, np.int64)
    for i in order:
        c = cnt[i]
        ok = (counts < WN) & ((loads + c) <= K).all(axis=1)
        if not ok.any():
            return None
        score = (loads + c).max(axis=1) * 1000 + counts
        score = np.where(ok, score, 1 << 40)
        w = int(np.argmin(score))
        assign[i] = w
        rank[i] = counts[w]
        counts[w] += 1
        loads[w] += c
    return assign, rank


def _wrap_idx(a):
    """[..., n] -> [..., 128, n//16]; idx i at [i%16, i//16], replicated x8."""
    lead = a.shape[:-1]
    n = a.shape[-1]
    w = a.reshape(*lead, n // 16, 16)
    w = np.moveaxis(w, -1, -2)
    return np.tile(w, (*([1] * len(lead)), 8, 1)).astype(np.int16)


def _wrap_col(a, dt):
    """[..., n] -> [..., 128, n//128]; slot p at [p%128, p//128]."""
    lead = a.shape[:-1]
    n = a.shape[-1]
    w = a.reshape(*lead, n // 128, 128)
    return np.ascontiguousarray(np.moveaxis(w, -1, -2)).astype(dt)


def _emit_layer(cfg, dl, gsrc, we, self_gidx, self_w, CH, assign, rank):
    """Emit per-core data streams for one layer.

    dl/gsrc/we: real edges (local dst, gather-space src index, weight).
    self_gidx/self_w: per natural-local-node self gather index and weight.
    Returns dict with gidx [128, NBATCH*GIDX_B] i16, dra/drb bf16, gw f32.
    """
    NB, K, NCOL, NBATCH = cfg.NB, cfg.K, cfg.NCOL, cfg.NBATCH
    IDX_REAL, IDX_SELF = cfg.IDX_REAL, cfg.IDX_SELF
    chunk = gsrc // CH
    w = assign[dl]
    r = rank[dl]
    key = w * 4 + chunk
    order = np.argsort(key, kind="stable")
    skey = key[order]
    cellcnt = np.bincount(skey, minlength=cfg.NWIN * 4)
    cellstart = np.concatenate([[0], np.cumsum(cellcnt)[:-1]])
    within = np.arange(len(order)) - cellstart[skey]
    assert within.max(initial=0) < K, f"cell overflow {within.max()} >= {K}"
    bb = w[order] // NB
    wl = w[order] % NB
    slot = wl * K + within
    ch = chunk[order]

    gidx = np.zeros((NBATCH, 4, IDX_REAL), np.int64)
    gwv = np.zeros((NBATCH, 4, IDX_REAL), np.float32)
    rk = np.full((NBATCH, 4, IDX_REAL), -1000.0, np.float32)
    gidx[bb, ch, slot] = gsrc[order] - ch * CH
    gwv[bb, ch, slot] = we[order]
    rk[bb, ch, slot] = r[order]

    slots = np.arange(IDX_REAL)
    cell_of = slots // K
    wlA = ((slots // 128) * 128) // K
    dra = np.where(cell_of[None, None, :] == wlA[None, None, :], rk, -1000.0)
    drb = np.where(cell_of[None, None, :] == (wlA + 1)[None, None, :], rk, -1000.0)
    # (dra/drb/gw emitted as f32 streams; consumed as tensor_scalar scalars)

    # self call
    node_at = np.full((cfg.NWIN, 128), -1, np.int64)
    node_at[assign, rank] = np.arange(len(assign))
    sidx = np.zeros((NBATCH, IDX_SELF), np.int64)
    sw = np.zeros((NBATCH, IDX_SELF), np.float32)
    sdr = np.full((NBATCH, IDX_SELF), -1000.0, np.float32)
    rr = np.arange(IDX_SELF) % 128
    for b in range(NBATCH):
        flat = node_at[b * NB:(b + 1) * NB].reshape(-1)
        valid = flat >= 0
        sidx[b][valid] = self_gidx[flat[valid]]
        sw[b][valid] = self_w[flat[valid]]
        sdr[b][valid] = rr[valid]

    # assemble streams: per batch: [call0..call3, self]
    gidx_parts, dra_parts, drb_parts, gw_parts = [], [], [], []
    for b in range(NBATCH):
        for c in range(4):
            gidx_parts.append(_wrap_idx(gidx[b, c]))
            dra_parts.append(_wrap_col(dra[b, c], np.float32))
            drb_parts.append(_wrap_col(drb[b, c], np.float32))
            gw_parts.append(_wrap_col(gwv[b, c], np.float32))
        gidx_parts.append(_wrap_idx(sidx[b]))
        dra_parts.append(_wrap_col(sdr[b], np.float32))
        gw_parts.append(_wrap_col(sw[b], np.float32))
    return {
        "gidx": np.concatenate(gidx_parts, axis=1),
        "dra": np.concatenate(dra_parts, axis=1),
        "drb": np.concatenate(drb_parts, axis=1),
        "gw": np.concatenate(gw_parts, axis=1),
    }


def prep(x, W1, b1, W2, b2, edge_index, cfg):
    """Full host prep. Returns (in_maps, perm2_list)."""
    N, R, NC = cfg.N, cfg.R, cfg.NC
    src = np.asarray(edge_index[0], np.int64)
    dst = np.asarray(edge_index[1], np.int64)
    x = np.asarray(x, np.float32)
    x_bf = x.astype(NPBF16)
    deg = np.ones(N, np.float32)
    np.add.at(deg, dst, 1.0)
    dinv = (1.0 / np.sqrt(deg)).astype(np.float32)

    owner = dst // R
    per_core = []
    for k in range(NC):
        m = owner == k
        per_core.append((src[m], dst[m] - k * R))

    # L1 packing
    pk1 = []
    for k in range(NC):
        s_k, d_k = per_core[k]
        cnt = np.zeros((R, 4), np.int64)
        np.add.at(cnt, (d_k, s_k // cfg.CH1), 1)
        r = _pack(cnt, cfg)
        assert r is not None, f"L1 packing failed core {k} (K={cfg.K})"
        pk1.append(r)

    cpos = np.empty(N, np.int64)
    for k in range(NC):
        a, rk = pk1[k]
        cpos[k * R:(k + 1) * R] = k * cfg.CROWS + a * 128 + rk

    # L2 packing
    pk2 = []
    for k in range(NC):
        s_k, d_k = per_core[k]
        cnt = np.zeros((R, 4), np.int64)
        np.add.at(cnt, (d_k, cpos[s_k] // cfg.CH2), 1)
        r = _pack(cnt, cfg)
        assert r is not None, f"L2 packing failed core {k} (K={cfg.K})"
        pk2.append(r)

    iota = np.tile(np.arange(128, dtype=np.float32), (128, 1)).astype(NPBF16)
    b1w = np.zeros((128, cfg.HID // 128), np.float32)
    for h in range(cfg.HID):
        b1w[h % 128, h // 128] = b1[h]
    b2bc = np.tile(np.asarray(b2, np.float32), (128, 1))

    in_maps = []
    perm2_list = []
    for k in range(NC):
        s_k, d_k = per_core[k]
        a1, r1 = pk1[k]
        a2, r2 = pk2[k]
        dloc = dinv[k * R:(k + 1) * R]

        st1 = _emit_layer(
            cfg, d_k, s_k, dinv[s_k] * dloc[d_k],
            np.arange(R), dloc * dloc, cfg.CH1, a1, r1)
        st2 = _emit_layer(
            cfg, d_k, cpos[s_k], dloc[d_k],
            a1 * 128 + r1, dloc, cfg.CH2, a2, r2)

        # dinv in L1-compact order (0 on pad rows)
        dc = np.zeros((cfg.NWIN, 128), np.float32)
        dc[a1, r1] = dloc
        perm2_list.append(a2 * 128 + r2)

        in_maps.append({
            "x": x_bf,
            "x_own": np.ascontiguousarray(x_bf[k * R:(k + 1) * R]),
            "W1": np.asarray(W1, np.float32),
            "W2": np.asarray(W2, np.float32),
            "b1w": b1w,
            "b2bc": b2bc,
            "iota": iota,
            "dinv1": np.ascontiguousarray(dc.T),
            "gidx1": st1["gidx"], "dra1": st1["dra"],
            "drb1": st1["drb"], "gw1": st1["gw"],
            "gidx2": st2["gidx"], "dra2": st2["dra"],
            "drb2": st2["drb"], "gw2": st2["gw"],
        })
    return in_maps, perm2_list


# ----------------------------------------------------------------------------
# device program
# ----------------------------------------------------------------------------

def _first_piece_per_bank(cfg, per_bank):
    """PSUM start=True must be issued exactly once per 2KB bank (the HW/sim
    zero-region granularity). Returns bank -> (c, j, lab, wl) of the first
    emitted aggregation matmul targeting that bank. Emission order:
    c in 0..3, j in 0..NCOL-1, A then B; self call last."""
    first = {}
    for c in range(4):
        for j, (a, split) in enumerate(cfg.colmap):
            for wl, lab in ((a, "A"), (a + 1, "B")) if split else ((a, "A"),):
                if wl >= cfg.NB:
                    continue
                bk = wl // per_bank
                if bk not in first:
                    first[bk] = (c, j, lab, wl)
    assert len(first) == (cfg.NB + per_bank - 1) // per_bank
    return first


def build_program(cfg, debug_taps=False):
    nc = bacc.Bacc(None, target_bir_lowering=False, debug=False)
    F, HID, C = cfg.F, cfg.HID, cfg.C
    NB, NCOL, NBATCH = cfg.NB, cfg.NCOL, cfg.NBATCH
    ddt = BF16 if cfg.dense_bf16 else F32

    x = nc.declare_dram_parameter("x", [cfg.N, F], BF16, isOutput=False)
    x_own = nc.declare_dram_parameter("x_own", [cfg.R, F], BF16, isOutput=False)
    W1 = nc.declare_dram_parameter("W1", [F, HID], F32, isOutput=False)
    W2 = nc.declare_dram_parameter("W2", [HID, C], F32, isOutput=False)
    b1w = nc.declare_dram_parameter("b1w", [128, HID // 128], F32, isOutput=False)
    b2bc = nc.declare_dram_parameter("b2bc", [128, C], F32, isOutput=False)
    iota = nc.declare_dram_parameter("iota", [128, 128], BF16, isOutput=False)
    dinv1 = nc.declare_dram_parameter("dinv1", [128, cfg.NWIN], F32, isOutput=False)
    gidx1 = nc.declare_dram_parameter("gidx1", [128, NBATCH * cfg.GIDX_B], I16, isOutput=False)
    dra1 = nc.declare_dram_parameter("dra1", [128, NBATCH * cfg.DRA_B], F32, isOutput=False)
    drb1 = nc.declare_dram_parameter("drb1", [128, NBATCH * cfg.DRB_B], F32, isOutput=False)
    gw1 = nc.declare_dram_parameter("gw1", [128, NBATCH * cfg.DRA_B], F32, isOutput=False)
    gidx2 = nc.declare_dram_parameter("gidx2", [128, NBATCH * cfg.GIDX_B], I16, isOutput=False)
    dra2 = nc.declare_dram_parameter("dra2", [128, NBATCH * cfg.DRA_B], F32, isOutput=False)
    drb2 = nc.declare_dram_parameter("drb2", [128, NBATCH * cfg.DRB_B], F32, isOutput=False)
    gw2 = nc.declare_dram_parameter("gw2", [128, NBATCH * cfg.DRA_B], F32, isOutput=False)
    outc = nc.declare_dram_parameter("outc", [cfg.CROWS, C], F32, isOutput=True)

    # h2s rows padded to F bf16 columns so the L2 gather elem is 256B-aligned;
    # pad columns are never read (matmul slices [:, 0:C]).
    h2s_c = nc.dram_tensor("h2s_c", [cfg.CROWS, F], BF16)
    h2s_full = nc.dram_tensor("h2s_full", [cfg.GC, F], BF16, addr_space="Shared")
    dbg_aggT = None
    if debug_taps:
        dbg_aggT = nc.declare_dram_parameter(
            "dbg_aggT", [128, cfg.CROWS], F32, isOutput=True)

    first_bank = _first_piece_per_bank(cfg, cfg.NB)

    with tile.TileContext(nc) as tc:
        with (
            tc.tile_pool(name="const", bufs=1) as pc,
            tc.tile_pool(name="gpool", bufs=2) as pg,
            tc.tile_pool(name="spool", bufs=2) as ps,
            tc.tile_pool(name="dense", bufs=2) as pd,
            tc.tile_pool(name="psagg", bufs=2, space="PSUM") as ppa,
            tc.tile_pool(name="psdense", bufs=2, space="PSUM") as ppd,
        ):
            # ---- load constants / streams into SBUF
            def load(param, shape, dtype, tag):
                t = pc.tile(shape, dtype, tag=tag)
                nc.sync.dma_start(out=t[:], in_=param[:, :])
                return t

            W1_sb = load(W1, [F, HID], F32, "W1sb")
            W2a_sb = pc.tile([128, C], F32, tag="W2a", name="W2a")
            W2b_sb = pc.tile([128, C], F32, tag="W2b", name="W2b")
            nc.sync.dma_start(out=W2a_sb[:], in_=W2[0:128, :])
            nc.sync.dma_start(out=W2b_sb[:], in_=W2[128:256, :])
            b1_sb = load(b1w, [128, HID // 128], F32, "b1sb")
            b2_sb = load(b2bc, [128, C], F32, "b2sb")
            iota_sb = load(iota, [128, 128], BF16, "iotasb")
            dinv1_sb = load(dinv1, [128, cfg.NWIN], F32, "dinv1sb")
            ident = pc.tile([128, 128], F32, tag="ident", name="ident")
            make_identity(nc, ident[:])

            if cfg.dense_bf16:
                W1_d = pc.tile([F, HID], BF16, tag="W1d", name="W1d")
                W2a_d = pc.tile([128, C], BF16, tag="W2ad", name="W2ad")
                W2b_d = pc.tile([128, C], BF16, tag="W2bd", name="W2bd")
                nc.vector.tensor_copy(W1_d[:], W1_sb[:])
                nc.vector.tensor_copy(W2a_d[:], W2a_sb[:])
                nc.vector.tensor_copy(W2b_d[:], W2b_sb[:])
            else:
                W1_d, W2a_d, W2b_d = W1_sb, W2a_sb, W2b_sb

            streams = {}
            for nm, par, wid, dt in (
                ("gidx1", gidx1, NBATCH * cfg.GIDX_B, I16),
                ("dra1", dra1, NBATCH * cfg.DRA_B, F32),
                ("drb1", drb1, NBATCH * cfg.DRB_B, F32),
                ("gw1", gw1, NBATCH * cfg.DRA_B, F32),
                ("gidx2", gidx2, NBATCH * cfg.GIDX_B, I16),
                ("dra2", dra2, NBATCH * cfg.DRA_B, F32),
                ("drb2", drb2, NBATCH * cfg.DRB_B, F32),
                ("gw2", gw2, NBATCH * cfg.DRA_B, F32),
            ):
                streams[nm] = load(par, [128, wid], dt, "st_" + nm)

            # ---- one layer of aggregation
            def emit_agg_layer(layer, elem, used, chunk_src_ap, self_src_ap, dense_fn):
                """used: number of leading elem-columns that carry real data
                (h2s rows are padded to `elem`; matmuls slice [:, 0:used])."""
                gi = streams[f"gidx{layer}"]
                da = streams[f"dra{layer}"]
                gw = streams[f"gw{layer}"]
                l1 = layer == 1
                nbatch = int(os.environ.get("GCN_NBATCH", str(NBATCH)))
                for b in range(nbatch):
                    go = b * cfg.GIDX_B
                    ao = b * cfg.DRA_B
                    # one psum bank per batch (NB windows side by side)
                    ww = 128 if l1 else C
                    bank = ppa.tile([128, NB * ww], F32, tag="aggps", name="aggps")

                    def wap(wl):
                        return bank[:, wl * ww:(wl + 1) * ww]

                    for c in range(4):
                        if os.environ.get("GCN_NO_GATHER"):
                            continue
                        G = pg.tile([128, NCOL, elem], BF16, tag="G", name="G")
                        nc.gpsimd.dma_gather(
                            G[:], chunk_src_ap(c),
                            gi[:, go + c * (cfg.IDX_REAL // 16):
                               go + (c + 1) * (cfg.IDX_REAL // 16)],
                            cfg.IDX_REAL, cfg.IDX_REAL, elem)
                        if os.environ.get("GCN_NO_S"):
                            continue
                        for j, (wa, split) in enumerate(cfg.colmap):
                            for wl, second in ((wa, False), (wa + 1, True)) if split else ((wa, False),):
                                if wl >= NB:
                                    continue
                                if os.environ.get("GCN_NO_MM"):
                                    continue
                                col = ao + c * NCOL + j
                                # S = onehot(dst_rank) * weight, one DVE op
                                S = ps.tile([128, 128], BF16, tag="S", name="S",
                                            bufs=6)
                                dsrc = streams[f"drb{layer}"] if second else da
                                nc.vector.tensor_scalar(
                                    S[:], iota_sb[:],
                                    dsrc[:, col:col + 1], gw[:, col:col + 1],
                                    OP.is_equal, OP.mult)
                                lab = "B" if second else "A"
                                start = first_bank[0] == (c, j, lab, wl)
                                if l1:
                                    nc.tensor.matmul(
                                        wap(wl), lhsT=G[:, j, 0:used], rhs=S[:],
                                        start=start, stop=False,
                                        skip_group_check=True)
                                else:
                                    nc.tensor.matmul(
                                        wap(wl), lhsT=S[:], rhs=G[:, j, 0:used],
                                        start=start, stop=False,
                                        skip_group_check=True)
                    # self call
                    if os.environ.get("GCN_NO_SELF"):
                        if not os.environ.get("GCN_NO_MM"):
                            dense_fn(b, bank)
                        continue
                    Gs = pg.tile([128, NB, elem], BF16, tag="Gs", name="Gs")
                    so = go + 4 * (cfg.IDX_REAL // 16)
                    nc.gpsimd.dma_gather(
                        Gs[:], self_src_ap,
                        gi[:, so: so + cfg.IDX_SELF // 16],
                        cfg.IDX_SELF, cfg.IDX_SELF, elem)
                    for wl in range(NB):
                        if os.environ.get("GCN_NO_MM"):
                            continue
                        col = ao + 4 * NCOL + wl
                        S = ps.tile([128, 128], BF16, tag="S", name="S", bufs=6)
                        nc.vector.tensor_scalar(
                            S[:], iota_sb[:],
                            da[:, col:col + 1], gw[:, col:col + 1],
                            OP.is_equal, OP.mult)
                        stop = wl == NB - 1
                        if l1:
                            nc.tensor.matmul(
                                wap(wl), lhsT=Gs[:, wl, 0:used], rhs=S[:],
                                start=False, stop=stop, skip_group_check=True)
                        else:
                            nc.tensor.matmul(
                                wap(wl), lhsT=S[:], rhs=Gs[:, wl, 0:used],
                                start=False, stop=stop, skip_group_check=True)
                    if not os.environ.get("GCN_NO_MM"):
                        dense_fn(b, bank)

            # ---- L1 dense tail: aggT [F, crows] -> h2s rows
            def dense_l1(b, bank):
                if os.environ.get("GCN_NO_DENSE"):
                    aggT_sb = pd.tile([128, NB * 128], ddt, tag="aggT", name="aggT")
                    nc.vector.tensor_copy(aggT_sb[:], bank[:])
                    return
                aggT_sb = pd.tile([128, NB * 128], ddt, tag="aggT", name="aggT")
                nc.vector.tensor_copy(aggT_sb[:], bank[:])
                if dbg_aggT is not None:
                    c0 = b * NB * 128
                    nc.sync.dma_start(
                        out=dbg_aggT[:, c0:c0 + NB * 128], in_=aggT_sb[:])
                hstage = pd.tile([128, NB, F], BF16, tag="hst", name="hst")
                nc.vector.memset(hstage[:], 0.0)
                for wl in range(NB):
                    w = b * NB + wl
                    a_sl = aggT_sb[:, wl * 128:(wl + 1) * 128]
                    ph = ppd.tile([128, 256], F32, tag="hT", name="hT")
                    nc.tensor.matmul(ph[:, 0:128], lhsT=W1_d[:, 0:128],
                                     rhs=a_sl, start=True, stop=False,
                                     skip_group_check=True)
                    nc.tensor.matmul(ph[:, 128:256], lhsT=W1_d[:, 128:256],
                                     rhs=a_sl, start=False, stop=True,
                                     skip_group_check=True)
                    hT = pd.tile([128, 256], ddt, tag="hTsb", name="hTsb")
                    nc.scalar.activation(hT[:, 0:128], ph[:, 0:128],
                                         AF.Relu, bias=b1_sb[:, 0:1])
                    nc.scalar.activation(hT[:, 128:256], ph[:, 128:256],
                                         AF.Relu, bias=b1_sb[:, 1:2])
                    p2 = ppd.tile([64, 128], F32, tag="h2T", name="h2T")
                    nc.tensor.matmul(p2[:], lhsT=W2a_d[:], rhs=hT[:, 0:128],
                                     start=True, stop=False,
                                     skip_group_check=True)
                    nc.tensor.matmul(p2[:], lhsT=W2b_d[:], rhs=hT[:, 128:256],
                                     start=False, stop=True,
                                     skip_group_check=True)
                    h2T = pd.tile([64, 128], F32, tag="h2Tsb", name="h2Tsb")
                    nc.vector.tensor_copy(h2T[:], p2[:])
                    pt = ppd.tile([128, 64], F32, tag="tp", name="tp")
                    nc.tensor.transpose(pt[:], h2T[:], ident[0:64, 0:64])
                    nc.vector.tensor_scalar(hstage[:, wl, 0:C], pt[:],
                                            dinv1_sb[:, w:w + 1], None, OP.mult)
                r0 = b * NB * 128
                nc.sync.dma_start(
                    out=h2s_c[r0:r0 + NB * 128, :]
                    .rearrange("(w r) f -> r w f", w=NB),
                    in_=hstage[:])

            def l1_chunk(c):
                return x[c * cfg.CH1:(c + 1) * cfg.CH1, :]


            # ---- L2 dense tail: psum [crows, C] -> +b2 -> exp/accum into
            # persistent staging; one batched Ln + final combine at the end
            # (keeps ACT on a single function per phase: table reloads are
            # ~1.3us each).
            xs_all = pc.tile([128, cfg.NWIN * C], F32, tag="xs_all", name="xs_all")
            nm_all = pc.tile([128, cfg.NWIN], F32, tag="nm_all", name="nm_all")
            ss_all = pc.tile([128, cfg.NWIN], F32, tag="ss_all", name="ss_all")

            def dense_l2(b, bank):
                for wl in range(NB):
                    w = b * NB + wl
                    xs = xs_all[:, w * C:(w + 1) * C]
                    nc.vector.tensor_add(xs, bank[:, wl * C:(wl + 1) * C], b2_sb[:])
                    nm = nm_all[:, w:w + 1]
                    nc.vector.tensor_reduce(nm, xs, mybir.AxisListType.X,
                                            OP.max, negate=True)
                    es = pd.tile([128, C], F32, tag="es", name="es")
                    nc.scalar.activation(es[:], xs, AF.Exp, bias=nm,
                                         accum_out=ss_all[:, w:w + 1])

            def final_l2():
                ls_all = pc.tile([128, cfg.NWIN], F32, tag="ls_all", name="ls_all")
                nc.scalar.activation(ls_all[:], ss_all[:], AF.Ln)
                for b in range(NBATCH):
                    ost = pd.tile([128, NB, C], F32, tag="ost", name="ost")
                    for wl in range(NB):
                        w = b * NB + wl
                        nc.vector.tensor_scalar(
                            ost[:, wl, :], xs_all[:, w * C:(w + 1) * C],
                            nm_all[:, w:w + 1], ls_all[:, w:w + 1],
                            OP.add, OP.subtract)
                    r0 = b * NB * 128
                    nc.sync.dma_start(
                        out=outc[r0:r0 + NB * 128, :]
                        .rearrange("(w r) f -> r w f", w=NB),
                        in_=ost[:])

            def l2_chunk(c):
                return h2s_full[c * cfg.CH2:(c + 1) * cfg.CH2, :]

            phase = int(os.environ.get("GCN_PHASE", "2"))
            repeat = int(os.environ.get("GCN_REPEAT", "1"))

            for _rep in range(repeat):
                emit_agg_layer(1, F, F, l1_chunk, x_own[:, :], dense_l1)

                if phase >= 1:
                    # ---- AllGather h2s
                    nc.gpsimd.collective_compute(
                        "AllGather", OP.bypass,
                        ins=[h2s_c[:, :]],
                        outs=[h2s_full[:, :]],
                        replica_groups=[list(range(cfg.NC))],
                    )
                if phase >= 2:
                    emit_agg_layer(2, F, C, l2_chunk, h2s_c[:, :], dense_l2)
                    final_l2()
            if phase < 2:
                # debug: copy h2s_c straight to outc and stop
                for rr in range(0, cfg.CROWS, 128):
                    tb = pd.tile([128, C], BF16, tag="cpb", name="cpb")
                    nc.sync.dma_start(out=tb[:], in_=h2s_c[rr:rr + 128, 0:C])
                    tt = pd.tile([128, C], F32, tag="cp", name="cp")
                    nc.vector.tensor_copy(tt[:], tb[:])
                    nc.sync.dma_start(out=outc[rr:rr + 128, :], in_=tt[:])


    nc.compile()
    return nc


_PROGRAM_CACHE = {}


def _get_program(cfg):
    key = (cfg.N, cfg.NWIN, cfg.NB, cfg.K, cfg.dense_bf16)
    if key not in _PROGRAM_CACHE:
        _PROGRAM_CACHE[key] = build_program(cfg)
    return _PROGRAM_CACHE[key]


def kernel(x, W1, b1, W2, b2, edge_index):
    cfg = CFG_FULL
    in_maps, perm2 = prep(x, W1, b1, W2, b2, edge_index, cfg)
    nc = _get_program(cfg)
    res = run_bass_kernel_spmd(
        nc, in_maps, core_ids=list(range(cfg.NC)),
        trace=bool(os.environ.get("GCN_TRACE")))
    if res.exec_time_ns is not None:
        print(f"HW exec time: {res.exec_time_ns} ns")
    out = np.empty((cfg.N, cfg.C), np.float32)
    for k in range(cfg.NC):
        out[k * cfg.R:(k + 1) * cfg.R] = res.results[k]["outc"][perm2[k]]
    return out



# revision 52
# speedup vs baseline: 2.3270x; 2.3270x over previous
"""GCN (2-layer, symmetric-normalized A+I) on 8 Trainium2 NeuronCores.

Strategy:
  L1 (dst-sharded, no comm): x is replicated, so core k aggregates x[src] for
  its own 12500 dst nodes via one-hot matmul segment-sum (bin-packed windows
  of 128 nodes x 4 src-chunk cells), then dense W1/relu/W2, scaling rows by
  dinv[d] into h2s_c (local compact order, rows padded to 256B for gather).

  L2 (src-sharded partials + ReduceScatter): core k owns h2 for its own nodes
  only.  It processes exactly the edges whose SRC it owns (plus self-loops of
  its own nodes), gathering from LOCAL h2s_c, and accumulates weighted
  one-hot matmul partial sums over a GLOBAL window layout shared by all cores
  (host packs all N nodes into W2 windows of <=128 nodes with per-core edge
  load <= 128).  The [W2*128, 64] bf16 partial buffers are then summed across
  cores with two chunked ReduceScatter(+add) collectives (output = 1/8th of
  the data => cheap), and each core finishes log_softmax on the compact rows
  it receives.  The host maps compact rows back to natural node order.

  Gathers are declared uint32 (256B rows = 64 x u32) and bitcast back to bf16
  for the PE, which halves the gather element count.

kernel(**inputs) takes FULL inputs and returns the FULL [100000, 64] output.
"""
import os
import numpy as np

os.environ.setdefault("NEURON_SCRATCHPAD_PAGE_SIZE", "256")

import concourse.bacc as bacc
import concourse.tile as tile
import concourse.mybir as mybir
from concourse import bass
from concourse.bass_utils import run_bass_kernel_spmd

F32 = mybir.dt.float32
BF16 = mybir.dt.bfloat16
I16 = mybir.dt.int16
U32 = mybir.dt.uint32
NPBF16 = mybir.dt.np(BF16)

AF = mybir.ActivationFunctionType
OP = mybir.AluOpType


class Cfg:
    def __init__(self, N=100000, R=12500, F=128, HID=256, C=64, NC=8,
                 NWIN1=100, NB1=4, K1=256, W2=800, NRS=2):
        self.N, self.R, self.F, self.HID, self.C, self.NC = N, R, F, HID, C, NC
        self.NWIN1, self.NB1, self.K1 = NWIN1, NB1, K1
        self.NBATCH1 = NWIN1 // NB1
        assert NB1 * K1 <= 1024 and (NB1 * K1) % 128 == 0
        self.NCOL1 = NB1 * K1 // 128
        self.IDX_REAL = NB1 * K1
        self.IDX_SELF = NB1 * 128
        self.CH1 = N // 4
        self.CROWS = NWIN1 * 128          # h2s rows per core
        assert self.CH1 <= 32768 and self.CROWS <= 32768
        # L1 column->cell map (no splits for K1=256)
        self.colmap = []
        for j in range(self.NCOL1):
            a = (j * 128) // K1
            self.colmap.append((a, (j * 128 + 127) // K1 != a))
        self.GIDX1_B = 4 * self.IDX_REAL // 16 + self.IDX_SELF // 16
        self.DRA1_B = 4 * self.NCOL1 + NB1
        self.DRB1_B = 4 * self.NCOL1
        # L2
        self.W2 = W2
        self.NB2 = 8
        self.K2 = 128
        assert W2 % (self.NB2 * NRS) == 0 and W2 % 16 == 0
        self.CALLS2 = W2 // self.NB2
        self.NRS = NRS
        # window split across RS chunks (large first, small last to shrink
        # the post-L2 tail); each chunk gets its own partial tensor
        self.WSPLIT = [W2 * 3 // 5, W2 - W2 * 3 // 5] if NRS == 2 else [W2]
        assert all(w % self.NB2 == 0 for w in self.WSPLIT)
        self.FWINS = [w * 128 // NC // 128 for w in self.WSPLIT]
        assert all(w * 128 % (NC * 128) == 0 for w in self.WSPLIT)
        self.FWIN_TOT = sum(self.FWINS)


CFG_FULL = Cfg()


# ----------------------------------------------------------------------------
# host prep
# ----------------------------------------------------------------------------

def _pack4(cnt, nwin, K, WN=128):
    """Bin-pack nodes (rows of cnt [n,4]) into nwin windows, per-cell cap K,
    count cap WN. Returns (assign, rank) or None."""
    degs = cnt.sum(1)
    order = np.argsort(-degs, kind="stable")
    loads = np.zeros((nwin, 4), np.int64)
    counts = np.zeros(nwin, np.int64)
    assign = np.full(len(degs), -1, np.int64)
    rank = np.full(len(degs), -1, np.int64)
    for i in order:
        c = cnt[i]
        ok = (counts < WN) & ((loads + c) <= K).all(axis=1)
        if not ok.any():
            return None
        score = (loads + c).max(axis=1) * 1000 + counts
        score = np.where(ok, score, 1 << 40)
        w = int(np.argmin(score))
        assign[i] = w
        rank[i] = counts[w]
        counts[w] += 1
        loads[w] += c
    return assign, rank


def _pack8(cnt8, W, K=128, WN=128):
    """Global 8-dim pack: all N nodes into W windows; per-core load cap K,
    count cap WN. Returns (assign, rank) or None."""
    n = len(cnt8)
    tot = cnt8.sum(1)
    order = np.argsort(-tot, kind="stable")
    loads = np.zeros((W, 8), np.int32)
    counts = np.zeros(W, np.int32)
    assign = np.full(n, -1, np.int64)
    rank = np.full(n, -1, np.int64)
    big = np.int64(1) << 40
    for i in order:
        c = cnt8[i]
        nl = loads + c[None, :]
        ok = (counts < WN) & (nl <= K).all(axis=1)
        if not ok.any():
            return None
        score = nl.max(axis=1).astype(np.int64) * 256 + counts
        score = np.where(ok, score, big)
        w = int(np.argmin(score))
        assign[i] = w
        rank[i] = counts[w]
        counts[w] += 1
        loads[w] = nl[w]
    return assign, rank


def _wrap_idx(a):
    """[..., n] -> [..., 128, n//16]; idx i at [i%16, i//16], replicated x8."""
    lead = a.shape[:-1]
    n = a.shape[-1]
    w = a.reshape(*lead, n // 16, 16)
    w = np.moveaxis(w, -1, -2)
    return np.tile(w, (*([1] * len(lead)), 8, 1)).astype(np.int16)


def _wrap_col(a, dt):
    """[..., n] -> [..., 128, n//128]; slot p at [p%128, p//128]."""
    lead = a.shape[:-1]
    n = a.shape[-1]
    w = a.reshape(*lead, n // 128, 128)
    return np.ascontiguousarray(np.moveaxis(w, -1, -2)).astype(dt)


def _emit_l1(cfg, dl, gsrc, we, self_w, assign, rank):
    """Per-core L1 streams. dl/gsrc/we: local dst, global src, edge weight.
    self gather index = local node id (into x_own); self_w per local node."""
    NB, K, NCOL, NBATCH = cfg.NB1, cfg.K1, cfg.NCOL1, cfg.NBATCH1
    IDX_REAL, IDX_SELF = cfg.IDX_REAL, cfg.IDX_SELF
    CH = cfg.CH1
    chunk = gsrc // CH
    w = assign[dl]
    r = rank[dl]
    key = w * 4 + chunk
    order = np.argsort(key, kind="stable")
    skey = key[order]
    cellcnt = np.bincount(skey, minlength=cfg.NWIN1 * 4)
    cellstart = np.concatenate([[0], np.cumsum(cellcnt)[:-1]])
    within = np.arange(len(order)) - cellstart[skey]
    assert within.max(initial=0) < K, f"L1 cell overflow {within.max()}"
    bb = w[order] // NB
    wl = w[order] % NB
    slot = wl * K + within
    ch = chunk[order]

    gidx = np.zeros((NBATCH, 4, IDX_REAL), np.int64)
    gwv = np.zeros((NBATCH, 4, IDX_REAL), np.float32)
    rk = np.full((NBATCH, 4, IDX_REAL), -1000.0, np.float32)
    gidx[bb, ch, slot] = gsrc[order] - ch * CH
    gwv[bb, ch, slot] = we[order]
    rk[bb, ch, slot] = r[order]

    slots = np.arange(IDX_REAL)
    cell_of = slots // K
    wlA = ((slots // 128) * 128) // K
    dra = np.where(cell_of[None, None, :] == wlA[None, None, :], rk, -1000.0)
    drb = np.where(cell_of[None, None, :] == (wlA + 1)[None, None, :], rk, -1000.0)

    # self call
    node_at = np.full((cfg.NWIN1, 128), -1, np.int64)
    node_at[assign, rank] = np.arange(len(assign))
    sidx = np.zeros((NBATCH, IDX_SELF), np.int64)
    sw = np.zeros((NBATCH, IDX_SELF), np.float32)
    sdr = np.full((NBATCH, IDX_SELF), -1000.0, np.float32)
    rr = np.arange(IDX_SELF) % 128
    for b in range(NBATCH):
        flat = node_at[b * NB:(b + 1) * NB].reshape(-1)
        valid = flat >= 0
        sidx[b][valid] = flat[valid]
        sw[b][valid] = self_w[flat[valid]]
        sdr[b][valid] = rr[valid]

    gidx_parts, dra_parts, drb_parts, gw_parts = [], [], [], []
    for b in range(NBATCH):
        for c in range(4):
            gidx_parts.append(_wrap_idx(gidx[b, c]))
            dra_parts.append(_wrap_col(dra[b, c], np.float32))
            drb_parts.append(_wrap_col(drb[b, c], np.float32))
            gw_parts.append(_wrap_col(gwv[b, c], np.float32))
        gidx_parts.append(_wrap_idx(sidx[b]))
        dra_parts.append(_wrap_col(sdr[b], np.float32))
        gw_parts.append(_wrap_col(sw[b], np.float32))
    return {
        "gidx1": np.concatenate(gidx_parts, axis=1),
        "dra1": np.concatenate(dra_parts, axis=1),
        "drb1": np.concatenate(drb_parts, axis=1),
        "gw1": np.concatenate(gw_parts, axis=1),
    }


def _emit_l2(cfg, dst_g, gsrc_local, we, assign2, rank2):
    """Per-core L2 streams over the GLOBAL window layout."""
    W2, K = cfg.W2, cfg.K2
    w = assign2[dst_g]
    r = rank2[dst_g]
    order = np.argsort(w, kind="stable")
    sw = w[order]
    cnt = np.bincount(sw, minlength=W2)
    start = np.concatenate([[0], np.cumsum(cnt)[:-1]])
    within = np.arange(len(order)) - start[sw]
    assert within.max(initial=0) < K, f"L2 cell overflow {within.max()}"
    slot = sw * K + within

    nslots = W2 * K
    gidx = np.zeros(nslots, np.int64)
    gwv = np.zeros(nslots, np.float32)
    rk = np.full(nslots, -1000.0, np.float32)
    gidx[slot] = gsrc_local[order]
    gwv[slot] = we[order]
    rk[slot] = r[order]

    ncalls = cfg.CALLS2
    per = cfg.NB2 * K
    return {
        "gidx2": np.concatenate(
            [_wrap_idx(gidx.reshape(ncalls, per)[i]) for i in range(ncalls)], axis=1),
        "dra2": np.concatenate(
            [_wrap_col(rk.reshape(ncalls, per)[i], np.float32) for i in range(ncalls)], axis=1),
        "gw2": np.concatenate(
            [_wrap_col(gwv.reshape(ncalls, per)[i], np.float32) for i in range(ncalls)], axis=1),
    }


def prep(x, W1, b1, W2, b2, edge_index, cfg):
    N, R, NC = cfg.N, cfg.R, cfg.NC
    src = np.asarray(edge_index[0], np.int64)
    dst = np.asarray(edge_index[1], np.int64)
    x_bf = np.ascontiguousarray(np.asarray(x, np.float32).astype(NPBF16))
    deg = np.ones(N, np.float32)
    np.add.at(deg, dst, 1.0)
    dinv = (1.0 / np.sqrt(deg)).astype(np.float32)

    # ---- L1: per-dst-owner edges
    owner_d = dst // R
    per_core_l1 = []
    for k in range(NC):
        m = owner_d == k
        per_core_l1.append((src[m], dst[m] - k * R))

    pk1 = []
    for k in range(NC):
        s_k, d_k = per_core_l1[k]
        cnt = np.zeros((R, 4), np.int64)
        np.add.at(cnt, (d_k, s_k // cfg.CH1), 1)
        r = _pack4(cnt, cfg.NWIN1, cfg.K1)
        assert r is not None, f"L1 packing failed core {k}"
        pk1.append(r)

    # local compact position of every node (within its owner's h2s_c,
    # r-major layout: row = rank * NWIN1 + window)
    cpos_local = np.empty(N, np.int64)
    for k in range(NC):
        a1, r1 = pk1[k]
        cpos_local[k * R:(k + 1) * R] = r1 * cfg.NWIN1 + a1

    # ---- L2: global window packing on per-src-owner dst counts (+ self)
    owner_s = src // R
    cnt8 = np.zeros((N, NC), np.int32)
    np.add.at(cnt8, (dst, owner_s), 1)
    cnt8[np.arange(N), np.arange(N) // R] += 1
    pk2 = None
    for W2G in (cfg.W2, cfg.W2 + 16, cfg.W2 + 32, cfg.W2 + 64):
        if W2G != cfg.W2:
            cfg = Cfg(W2=W2G)
        pk2 = _pack8(cnt8, cfg.W2, cfg.K2)
        if pk2 is not None:
            break
    assert pk2 is not None, "L2 global packing failed"
    a2, r2 = pk2

    iota = np.tile(np.arange(128, dtype=np.float32), (128, 1)).astype(NPBF16)
    b1w = np.zeros((128, cfg.HID // 128), np.float32)
    for h in range(cfg.HID):
        b1w[h % 128, h // 128] = b1[h]
    b2bc = np.tile(np.asarray(b2, np.float32), (128, 8)).astype(NPBF16)

    in_maps = []
    for k in range(NC):
        s_k, d_k = per_core_l1[k]
        a1, r1 = pk1[k]
        dloc = dinv[k * R:(k + 1) * R]

        st1 = _emit_l1(cfg, d_k, s_k, dinv[s_k] * dloc[d_k], dloc * dloc, a1, r1)

        # L2 edges: src owned by k (+ self loops of k's nodes)
        m = owner_s == k
        l2_dst = np.concatenate([dst[m], np.arange(k * R, (k + 1) * R)])
        l2_srcl = np.concatenate([cpos_local[src[m]], cpos_local[k * R:(k + 1) * R]])
        l2_we = dinv[np.concatenate([dst[m], np.arange(k * R, (k + 1) * R)])]
        st2 = _emit_l2(cfg, l2_dst, l2_srcl, l2_we, a2, r2)

        dc = np.zeros((cfg.NWIN1, 128), np.float32)
        dc[a1, r1] = dloc

        in_maps.append({
            "x": x_bf,
            "x_own": np.ascontiguousarray(x_bf[k * R:(k + 1) * R]),
            "W1": np.asarray(W1, np.float32),
            "W2w": np.asarray(W2, np.float32),
            "b1w": b1w,
            "b2bc": b2bc,
            "iota": iota,
            "dinv1": np.ascontiguousarray(dc.T),
            **st1,
            **st2,
        })

    # host map: node -> (owner core, outc partition p, outc column wcol)
    # partial_j is r-major [128, Wj, C] over its window subrange; RS chunk j
    # scatters flat (rank, window) rows; core k gets ranks [16k, 16k+16).
    wsplit = np.asarray(cfg.WSPLIT)
    wstart = np.concatenate([[0], np.cumsum(wsplit)[:-1]])
    fstart = np.concatenate([[0], np.cumsum(cfg.FWINS)[:-1]])
    j = np.searchsorted(np.cumsum(wsplit), a2, side="right")
    wj = a2 - wstart[j]
    RPC = 128 // cfg.NC
    owner = r2 // RPC
    i = (r2 % RPC) * wsplit[j] + wj
    p = i % 128
    wcol = fstart[j] + i // 128
    return cfg, in_maps, owner, p, wcol


# ----------------------------------------------------------------------------
# device program
# ----------------------------------------------------------------------------

def build_program(cfg):
    nc = bacc.Bacc(None, target_bir_lowering=False, debug=False)
    F, HID, C = cfg.F, cfg.HID, cfg.C
    NB1, NCOL1, NBATCH1 = cfg.NB1, cfg.NCOL1, cfg.NBATCH1

    x = nc.declare_dram_parameter("x", [cfg.N, F], BF16, isOutput=False)
    x_own = nc.declare_dram_parameter("x_own", [cfg.R, F], BF16, isOutput=False)
    W1p = nc.declare_dram_parameter("W1", [F, HID], F32, isOutput=False)
    W2p = nc.declare_dram_parameter("W2w", [HID, C], F32, isOutput=False)
    b1w = nc.declare_dram_parameter("b1w", [128, HID // 128], F32, isOutput=False)
    b2bc = nc.declare_dram_parameter("b2bc", [128, 8 * C], BF16, isOutput=False)
    iota = nc.declare_dram_parameter("iota", [128, 128], BF16, isOutput=False)
    dinv1 = nc.declare_dram_parameter("dinv1", [128, cfg.NWIN1], F32, isOutput=False)
    gidx1 = nc.declare_dram_parameter("gidx1", [128, NBATCH1 * cfg.GIDX1_B], I16, isOutput=False)
    dra1 = nc.declare_dram_parameter("dra1", [128, NBATCH1 * cfg.DRA1_B], F32, isOutput=False)
    drb1 = nc.declare_dram_parameter("drb1", [128, NBATCH1 * cfg.DRB1_B], F32, isOutput=False)
    gw1 = nc.declare_dram_parameter("gw1", [128, NBATCH1 * cfg.DRA1_B], F32, isOutput=False)
    gidx2 = nc.declare_dram_parameter("gidx2", [128, cfg.CALLS2 * 64], I16, isOutput=False)
    dra2 = nc.declare_dram_parameter("dra2", [128, cfg.W2], F32, isOutput=False)
    gw2 = nc.declare_dram_parameter("gw2", [128, cfg.W2], F32, isOutput=False)
    outc = nc.declare_dram_parameter(
        "outc", [128, cfg.FWIN_TOT, C], F32, isOutput=True)

    # r-major 3D layouts so staging DMAs are contiguous per partition
    h2s_c = nc.dram_tensor("h2s_c", [128, cfg.NWIN1, F], BF16)
    partials = [nc.dram_tensor(f"partial{j}", [128, w, C], BF16)
                for j, w in enumerate(cfg.WSPLIT)]
    rs_out = [nc.dram_tensor(f"rs_out{j}", [fw * 128, C], BF16)
              for j, fw in enumerate(cfg.FWINS)]

    with tile.TileContext(nc) as tc:
        with (
            tc.tile_pool(name="const", bufs=1) as pc,
            tc.tile_pool(name="gpool", bufs=3) as pg,
            tc.tile_pool(name="spool", bufs=2) as ps,
            tc.tile_pool(name="dense", bufs=2) as pd,
            tc.tile_pool(name="psagg", bufs=2, space="PSUM") as ppa,
            tc.tile_pool(name="psdense", bufs=2, space="PSUM") as ppd,
            tc.tile_pool(name="psl2", bufs=2, space="PSUM") as ppl,
        ):
            def load(param, shape, dtype, tag):
                t = pc.tile(shape, dtype, tag=tag)
                nc.sync.dma_start(out=t[:], in_=param[:, :])
                return t

            # L1-critical streams first so the first gathers start ASAP
            streams = {}
            for nm, par, wid, dt in (
                ("gidx1", gidx1, NBATCH1 * cfg.GIDX1_B, I16),
                ("dra1", dra1, NBATCH1 * cfg.DRA1_B, F32),
                ("gw1", gw1, NBATCH1 * cfg.DRA1_B, F32),
            ):
                streams[nm] = load(par, [128, wid], dt, "st_" + nm)
            iota_sb = load(iota, [128, 128], BF16, "iotasb")
            W1f = load(W1p, [F, HID], F32, "W1f")
            W2fa = pc.tile([128, C], F32, tag="W2fa")
            W2fb = pc.tile([128, C], F32, tag="W2fb")
            nc.sync.dma_start(out=W2fa[:], in_=W2p[0:128, :])
            nc.sync.dma_start(out=W2fb[:], in_=W2p[128:256, :])
            b1_sb = load(b1w, [128, HID // 128], F32, "b1sb")
            b2_sb = load(b2bc, [128, 8 * C], BF16, "b2sb")
            dinv1_sb = load(dinv1, [128, cfg.NWIN1], F32, "dinv1sb")

            W1_d = pc.tile([F, HID], BF16, tag="W1d")
            nc.vector.tensor_copy(W1_d[:], W1f[:])
            W2a_d = pc.tile([128, C], BF16, tag="W2ad")
            W2b_d = pc.tile([128, C], BF16, tag="W2bd")
            nc.vector.tensor_copy(W2a_d[:], W2fa[:])
            nc.vector.tensor_copy(W2b_d[:], W2fb[:])

            for nm, par, wid, dt in (
                ("drb1", drb1, NBATCH1 * cfg.DRB1_B, F32),
                ("gidx2", gidx2, cfg.CALLS2 * 64, I16),
                ("dra2", dra2, cfg.W2, F32),
                ("gw2", gw2, cfg.W2, F32),
            ):
                streams[nm] = load(par, [128, wid], dt, "st_" + nm)

            # two persistent h2s staging tiles, pad columns zeroed once
            hstages = []
            for i in range(2):
                t = pc.tile([128, NB1, F], BF16, tag=f"hst{i}")
                nc.vector.memset(t[:], 0.0)
                hstages.append(t)

            # pin the one act table containing Relu/Copy/Exp/Ln so no
            # mid-kernel table reloads happen
            warm = pc.tile([128, 1], F32, tag="warm")
            nc.scalar.activation(warm[:], b1_sb[:, 0:1], AF.Ln)
            nc.scalar.activation(warm[:], b1_sb[:, 0:1], AF.Exp)

            # ---------------- L1 ----------------
            gi1, da1, db1, gwt1 = (streams["gidx1"], streams["dra1"],
                                   streams["drb1"], streams["gw1"])

            def dense_l1(b, bank):
                aggT = pd.tile([128, NB1 * 128], BF16, tag="aggT")
                nc.scalar.activation(aggT[:], bank[:], AF.Copy)
                hstage = hstages[b % 2]
                for wl in range(NB1):
                    w = b * NB1 + wl
                    a_sl = aggT[:, wl * 128:(wl + 1) * 128]
                    ph = ppd.tile([128, HID], F32, tag="ph")
                    nc.tensor.matmul(ph[:, 0:128], lhsT=W1_d[:, 0:128],
                                     rhs=a_sl, start=True, stop=False,
                                     skip_group_check=True)
                    nc.tensor.matmul(ph[:, 128:256], lhsT=W1_d[:, 128:256],
                                     rhs=a_sl, start=False, stop=True,
                                     skip_group_check=True)
                    hT = pd.tile([128, HID], BF16, tag="hT")
                    nc.scalar.activation(hT[:, 0:128], ph[:, 0:128],
                                         AF.Relu, bias=b1_sb[:, 0:1])
                    nc.scalar.activation(hT[:, 128:256], ph[:, 128:256],
                                         AF.Relu, bias=b1_sb[:, 1:2])
                    p2 = ppd.tile([128, C], F32, tag="p2")
                    nc.tensor.matmul(p2[:], lhsT=hT[:, 0:128], rhs=W2a_d[:],
                                     start=True, stop=False,
                                     skip_group_check=True)
                    nc.tensor.matmul(p2[:], lhsT=hT[:, 128:256], rhs=W2b_d[:],
                                     start=False, stop=True,
                                     skip_group_check=True)
                    nc.vector.tensor_scalar(hstage[:, wl, 0:C], p2[:],
                                            dinv1_sb[:, w:w + 1], None, OP.mult)
                nc.sync.dma_start(
                    out=h2s_c[:, b * NB1:(b + 1) * NB1, :], in_=hstage[:])

            for b in range(NBATCH1):
                go = b * cfg.GIDX1_B
                ao = b * cfg.DRA1_B
                bank = ppa.tile([128, NB1 * 128], F32, tag="aggps")

                first = True
                for c in range(4):
                    G = pg.tile([128, NCOL1, F // 2], U32, tag="G1")
                    nc.gpsimd.dma_gather(
                        G[:], x[c * cfg.CH1:(c + 1) * cfg.CH1, :].bitcast(U32),
                        gi1[:, go + c * (cfg.IDX_REAL // 16):
                            go + (c + 1) * (cfg.IDX_REAL // 16)],
                        cfg.IDX_REAL, cfg.IDX_REAL, F // 2)
                    Gbf = G[:].bitcast(BF16)
                    for j, (wa, split) in enumerate(cfg.colmap):
                        for wl, second in ((wa, False), (wa + 1, True)) if split else ((wa, False),):
                            if wl >= NB1:
                                continue
                            col = ao + c * NCOL1 + j
                            S = ps.tile([128, 128], BF16, tag="S", bufs=6)
                            dsrc = db1 if second else da1
                            dcol = (b * cfg.DRB1_B + c * NCOL1 + j) if second else col
                            eng = nc.gpsimd if c == 3 else nc.vector
                            eng.tensor_scalar(
                                S[:], iota_sb[:],
                                dsrc[:, dcol:dcol + 1], gwt1[:, col:col + 1],
                                OP.is_equal, OP.mult)
                            nc.tensor.matmul(
                                bank[:, wl * 128:(wl + 1) * 128],
                                lhsT=Gbf[:, j, :], rhs=S[:],
                                start=first, stop=False, skip_group_check=True)
                            first = False
                # self
                Gs = pg.tile([128, NB1, F // 2], U32, tag="Gs1")
                so = go + 4 * (cfg.IDX_REAL // 16)
                nc.gpsimd.dma_gather(
                    Gs[:], x_own[:, :].bitcast(U32),
                    gi1[:, so:so + cfg.IDX_SELF // 16],
                    cfg.IDX_SELF, cfg.IDX_SELF, F // 2)
                Gsbf = Gs[:].bitcast(BF16)
                for wl in range(NB1):
                    col = ao + 4 * NCOL1 + wl
                    S = ps.tile([128, 128], BF16, tag="S", bufs=6)
                    nc.gpsimd.tensor_scalar(
                        S[:], iota_sb[:],
                        da1[:, col:col + 1], gwt1[:, col:col + 1],
                        OP.is_equal, OP.mult)
                    nc.tensor.matmul(
                        bank[:, wl * 128:(wl + 1) * 128],
                        lhsT=Gsbf[:, wl, :], rhs=S[:],
                        start=False, stop=(wl == NB1 - 1),
                        skip_group_check=True)
                dense_l1(b, bank)

            # ---------------- L2: src-sharded partials ----------------
            gi2, da2, gwt2 = streams["gidx2"], streams["dra2"], streams["gw2"]
            NB2, K2 = cfg.NB2, cfg.K2
            call_split = [w // NB2 for w in cfg.WSPLIT]

            last_gather = None
            for call in range(cfg.CALLS2):
                G2 = pg.tile([128, NB2, F // 2], U32, tag="G2", bufs=24)
                last_gather = nc.gpsimd.dma_gather(
                    G2[:],
                    h2s_c[:, :, :].rearrange("r w f -> (r w) f").bitcast(U32),
                    gi2[:, call * 64:(call + 1) * 64],
                    NB2 * K2, NB2 * K2, F // 2)
                G2bf = G2[:].bitcast(BF16)
                bank2 = ppl.tile([128, NB2 * C], F32, tag="l2ps")
                for wl in range(NB2):
                    w = call * NB2 + wl
                    S = ps.tile([128, 128], BF16, tag="S2", bufs=12)
                    eng = nc.gpsimd if wl < 2 else nc.vector
                    si = eng.tensor_scalar(
                        S[:], iota_sb[:],
                        da2[:, w:w + 1], gwt2[:, w:w + 1],
                        OP.is_equal, OP.mult)
                    if wl < 2:
                        last_gather = si
                    nc.tensor.matmul(
                        bank2[:, wl * C:(wl + 1) * C],
                        lhsT=S[:], rhs=G2bf[:, wl, 0:C],
                        start=(wl == 0), stop=(wl == NB2 - 1),
                        skip_group_check=True)
                stage = pd.tile([128, NB2, C], BF16, tag="pstage", bufs=4)
                nc.scalar.activation(stage[:], bank2[:], AF.Copy)
                pj = 0 if call < call_split[0] else 1
                pcall = call - (0 if pj == 0 else call_split[0])
                nc.sync.dma_start(
                    out=partials[pj][:, pcall * NB2:(pcall + 1) * NB2, :],
                    in_=stage[:])

            # Collectives and the rs_out loads all live on the Pool queue,
            # explicitly chained (RS0 -> lt0 -> RS1 -> lt1) and pinned after
            # the last gather: the tile scheduler otherwise hoists the
            # collectives and stalls the L2 pipeline behind them.
            from concourse.bass import _add_dep_helper
            lts = []
            prev = last_gather
            for j in range(cfg.NRS):
                cc = nc.gpsimd.collective_compute(
                    "ReduceScatter", OP.add,
                    ins=[partials[j][:, :, :]],
                    outs=[rs_out[j][:, :]],
                    replica_groups=[list(range(cfg.NC))],
                )
                _add_dep_helper(cc.ins, prev.ins, sync=True,
                                reason="keep Pool queue order")
                FW = cfg.FWINS[j]
                lt = pd.tile([128, FW, C], BF16, tag=f"ltall{j}", bufs=1)
                ld = nc.gpsimd.dma_start(
                    out=lt[:],
                    in_=rs_out[j][:, :].rearrange("(w r) c -> r w c", w=FW))
                lts.append(lt)
                prev = ld

            # ---------------- final: +b2, log_softmax on received rows ----
            # (all on DVE/ACT/SP: Pool is busy with the collectives)
            fcol = 0
            for j in range(cfg.NRS):
                FW = cfg.FWINS[j]
                lt = lts[j]
                # logits are O(+-40) so exp cannot overflow f32: skip the
                # usual max-shift and compute log_softmax = x - ln(sum(exp x)),
                # batching 8 windows per op
                xs_all = pc.tile([128, FW * C], F32, tag=f"xs{j}")
                ss_all = pc.tile([128, FW], F32, tag=f"ss{j}")
                for g0 in range(0, FW, 8):
                    gn = min(8, FW - g0)
                    xs = xs_all[:, g0 * C:(g0 + gn) * C]
                    nc.vector.tensor_add(
                        xs, lt[:, g0:g0 + gn, :], b2_sb[:, 0:gn * C])
                    es = pd.tile([128, 8 * C], F32, tag="es")
                    nc.scalar.activation(es[:, 0:gn * C], xs, AF.Exp)
                    nc.vector.tensor_reduce(
                        ss_all[:, g0:g0 + gn],
                        es[:, 0:gn * C].rearrange("p (g c) -> p g c", g=gn),
                        mybir.AxisListType.X, OP.add)
                ls_all = pc.tile([128, FW], F32, tag=f"ls{j}")
                nc.scalar.activation(ls_all[:], ss_all[:], AF.Ln)
                ost = pd.tile([128, FW, C], F32, tag=f"ost{j}", bufs=1)
                for fw in range(FW):
                    nc.vector.tensor_scalar(
                        ost[:, fw, :], xs_all[:, fw * C:(fw + 1) * C],
                        ls_all[:, fw:fw + 1], None, OP.subtract)
                nc.sync.dma_start(
                    out=outc[:, fcol:fcol + FW, :], in_=ost[:])
                fcol += FW

    nc.compile()
    return nc


_PROGRAM_CACHE = {}


def _get_program(cfg):
    key = (cfg.N, cfg.NWIN1, cfg.NB1, cfg.K1, cfg.W2, cfg.NRS)
    if key not in _PROGRAM_CACHE:
        _PROGRAM_CACHE[key] = build_program(cfg)
    return _PROGRAM_CACHE[key]


def kernel(x, W1, b1, W2, b2, edge_index):
    cfg, in_maps, owner, p, wcol = prep(x, W1, b1, W2, b2, edge_index, CFG_FULL)
    nc = _get_program(cfg)
    res = run_bass_kernel_spmd(
        nc, in_maps, core_ids=list(range(cfg.NC)),
        trace=bool(os.environ.get("GCN_TRACE")))
    if res.exec_time_ns is not None:
        print(f"HW exec time: {res.exec_time_ns} ns")
    outs = np.stack([res.results[k]["outc"] for k in range(cfg.NC)])
    return outs[owner, p, wcol].astype(np.float32)


# revision 64
# speedup vs baseline: 2.5249x; 1.0850x over previous
"""GCN (2-layer, symmetric-normalized A+I) on 8 Trainium2 NeuronCores.

Strategy:
  L1 (dst-sharded, no comm): x is replicated, so core k aggregates x[src] for
  its own 12500 dst nodes via one-hot matmul segment-sum (bin-packed windows
  of 128 nodes x 4 src-chunk cells), then dense W1/relu/W2, scaling rows by
  dinv[d] into h2s_c (local compact order, rows padded to 256B for gather).

  L2 (src-sharded partials + ReduceScatter): core k owns h2 for its own nodes
  only.  It processes exactly the edges whose SRC it owns (plus self-loops of
  its own nodes), gathering from LOCAL h2s_c, and accumulates weighted
  one-hot matmul partial sums over a GLOBAL window layout shared by all cores
  (host packs all N nodes into W2 windows of <=128 nodes with per-core edge
  load <= 128).  The [W2*128, 64] bf16 partial buffers are then summed across
  cores with two chunked ReduceScatter(+add) collectives (output = 1/8th of
  the data => cheap), and each core finishes log_softmax on the compact rows
  it receives.  The host maps compact rows back to natural node order.

  Gathers are declared uint32 (256B rows = 64 x u32) and bitcast back to bf16
  for the PE, which halves the gather element count.

kernel(**inputs) takes FULL inputs and returns the FULL [100000, 64] output.
"""
import os
import numpy as np

os.environ.setdefault("NEURON_SCRATCHPAD_PAGE_SIZE", "256")

import concourse.bacc as bacc
import concourse.tile as tile
import concourse.mybir as mybir
from concourse import bass
from concourse.bass_utils import run_bass_kernel_spmd

F32 = mybir.dt.float32
BF16 = mybir.dt.bfloat16
I16 = mybir.dt.int16
U32 = mybir.dt.uint32
U64 = mybir.dt.uint64
NPBF16 = mybir.dt.np(BF16)

AF = mybir.ActivationFunctionType
OP = mybir.AluOpType


class Cfg:
    def __init__(self, N=100000, R=12500, F=128, HID=256, C=64, NC=8,
                 NWIN1=100, NB1=4, K1=256, W2=784, NRS=2):
        self.N, self.R, self.F, self.HID, self.C, self.NC = N, R, F, HID, C, NC
        self.NWIN1, self.NB1, self.K1 = NWIN1, NB1, K1
        self.NBATCH1 = NWIN1 // NB1
        assert NB1 * K1 <= 1024 and (NB1 * K1) % 128 == 0
        self.NCOL1 = NB1 * K1 // 128
        self.IDX_REAL = NB1 * K1
        self.IDX_SELF = NB1 * 128
        self.CH1 = N // 4
        self.CROWS = NWIN1 * 128          # h2s rows per core
        assert self.CH1 <= 32768 and self.CROWS <= 32768
        # L1 column->cell map (no splits for K1=256)
        self.colmap = []
        for j in range(self.NCOL1):
            a = (j * 128) // K1
            self.colmap.append((a, (j * 128 + 127) // K1 != a))
        self.GIDX1_B = 4 * self.IDX_REAL // 16 + self.IDX_SELF // 16
        self.DRA1_B = 4 * self.NCOL1 + NB1
        self.DRB1_B = 4 * self.NCOL1
        # L2
        self.W2 = W2
        self.NB2 = 8
        self.K2 = 128
        assert W2 % (self.NB2 * NRS) == 0 and W2 % 16 == 0
        self.CALLS2 = W2 // self.NB2
        self.NRS = NRS
        # window split across RS chunks (large first, small last to shrink
        # the post-L2 tail); each chunk gets its own partial tensor
        big = (W2 * 3 // 4 + 7) // 8 * 8
        self.WSPLIT = [big, W2 - big] if NRS == 2 else [W2]
        assert all(w % self.NB2 == 0 for w in self.WSPLIT)
        self.FWINS = [w * 128 // NC // 128 for w in self.WSPLIT]
        assert all(w * 128 % (NC * 128) == 0 for w in self.WSPLIT)
        self.FWIN_TOT = sum(self.FWINS)


CFG_FULL = Cfg()


# ----------------------------------------------------------------------------
# host prep
# ----------------------------------------------------------------------------

def _pack4(cnt, nwin, K, WN=128):
    """Bin-pack nodes (rows of cnt [n,4]) into nwin windows, per-cell cap K,
    count cap WN. Returns (assign, rank) or None."""
    degs = cnt.sum(1)
    order = np.argsort(-degs, kind="stable")
    loads = np.zeros((nwin, 4), np.int64)
    counts = np.zeros(nwin, np.int64)
    assign = np.full(len(degs), -1, np.int64)
    rank = np.full(len(degs), -1, np.int64)
    for i in order:
        c = cnt[i]
        ok = (counts < WN) & ((loads + c) <= K).all(axis=1)
        if not ok.any():
            return None
        score = (loads + c).max(axis=1) * 1000 + counts
        score = np.where(ok, score, 1 << 40)
        w = int(np.argmin(score))
        assign[i] = w
        rank[i] = counts[w]
        counts[w] += 1
        loads[w] += c
    return assign, rank


def _pack8(cnt8, W, K=128, WN=128):
    """Global 8-dim pack: all N nodes into W windows; per-core load cap K,
    count cap WN. Returns (assign, rank) or None."""
    n = len(cnt8)
    tot = cnt8.sum(1)
    order = np.argsort(-tot, kind="stable")
    loads = np.zeros((W, 8), np.int32)
    counts = np.zeros(W, np.int32)
    assign = np.full(n, -1, np.int64)
    rank = np.full(n, -1, np.int64)
    big = np.int64(1) << 40
    for i in order:
        c = cnt8[i]
        nl = loads + c[None, :]
        ok = (counts < WN) & (nl <= K).all(axis=1)
        if not ok.any():
            return None
        score = nl.max(axis=1).astype(np.int64) * 256 + counts
        score = np.where(ok, score, big)
        w = int(np.argmin(score))
        assign[i] = w
        rank[i] = counts[w]
        counts[w] += 1
        loads[w] = nl[w]
    return assign, rank


def _wrap_idx(a):
    """[..., n] -> [..., 128, n//16]; idx i at [i%16, i//16], replicated x8."""
    lead = a.shape[:-1]
    n = a.shape[-1]
    w = a.reshape(*lead, n // 16, 16)
    w = np.moveaxis(w, -1, -2)
    return np.tile(w, (*([1] * len(lead)), 8, 1)).astype(np.int16)


def _wrap_col(a, dt):
    """[..., n] -> [..., 128, n//128]; slot p at [p%128, p//128]."""
    lead = a.shape[:-1]
    n = a.shape[-1]
    w = a.reshape(*lead, n // 128, 128)
    return np.ascontiguousarray(np.moveaxis(w, -1, -2)).astype(dt)


def _emit_l1(cfg, dl, gsrc, we, self_w, assign, rank):
    """Per-core L1 streams. dl/gsrc/we: local dst, global src, edge weight.
    self gather index = local node id (into x_own); self_w per local node."""
    NB, K, NCOL, NBATCH = cfg.NB1, cfg.K1, cfg.NCOL1, cfg.NBATCH1
    IDX_REAL, IDX_SELF = cfg.IDX_REAL, cfg.IDX_SELF
    CH = cfg.CH1
    chunk = gsrc // CH
    w = assign[dl]
    r = rank[dl]
    key = w * 4 + chunk
    order = np.argsort(key, kind="stable")
    skey = key[order]
    cellcnt = np.bincount(skey, minlength=cfg.NWIN1 * 4)
    cellstart = np.concatenate([[0], np.cumsum(cellcnt)[:-1]])
    within = np.arange(len(order)) - cellstart[skey]
    assert within.max(initial=0) < K, f"L1 cell overflow {within.max()}"
    bb = w[order] // NB
    wl = w[order] % NB
    slot = wl * K + within
    ch = chunk[order]

    gidx = np.zeros((NBATCH, 4, IDX_REAL), np.int64)
    gwv = np.zeros((NBATCH, 4, IDX_REAL), np.float32)
    rk = np.full((NBATCH, 4, IDX_REAL), -1000.0, np.float32)
    gidx[bb, ch, slot] = gsrc[order] - ch * CH
    gwv[bb, ch, slot] = we[order]
    rk[bb, ch, slot] = r[order]

    slots = np.arange(IDX_REAL)
    cell_of = slots // K
    wlA = ((slots // 128) * 128) // K
    dra = np.where(cell_of[None, None, :] == wlA[None, None, :], rk, -1000.0)
    drb = np.where(cell_of[None, None, :] == (wlA + 1)[None, None, :], rk, -1000.0)

    # self call
    node_at = np.full((cfg.NWIN1, 128), -1, np.int64)
    node_at[assign, rank] = np.arange(len(assign))
    sidx = np.zeros((NBATCH, IDX_SELF), np.int64)
    sw = np.zeros((NBATCH, IDX_SELF), np.float32)
    sdr = np.full((NBATCH, IDX_SELF), -1000.0, np.float32)
    rr = np.arange(IDX_SELF) % 128
    for b in range(NBATCH):
        flat = node_at[b * NB:(b + 1) * NB].reshape(-1)
        valid = flat >= 0
        sidx[b][valid] = flat[valid]
        sw[b][valid] = self_w[flat[valid]]
        sdr[b][valid] = rr[valid]

    gidx_parts, dra_parts, drb_parts, gw_parts = [], [], [], []
    for b in range(NBATCH):
        for c in range(4):
            gidx_parts.append(_wrap_idx(gidx[b, c]))
            dra_parts.append(_wrap_col(dra[b, c], np.float32))
            drb_parts.append(_wrap_col(drb[b, c], np.float32))
            gw_parts.append(_wrap_col(gwv[b, c], np.float32))
        gidx_parts.append(_wrap_idx(sidx[b]))
        dra_parts.append(_wrap_col(sdr[b], np.float32))
        gw_parts.append(_wrap_col(sw[b], np.float32))
    return {
        "gidx1": np.concatenate(gidx_parts, axis=1),
        "dra1": np.concatenate(dra_parts, axis=1),
        "drb1": np.concatenate(drb_parts, axis=1),
        "gw1": np.concatenate(gw_parts, axis=1),
    }


def _emit_l2(cfg, dst_g, gsrc_local, we, assign2, rank2):
    """Per-core L2 streams over the GLOBAL window layout."""
    W2, K = cfg.W2, cfg.K2
    w = assign2[dst_g]
    r = rank2[dst_g]
    order = np.argsort(w, kind="stable")
    sw = w[order]
    cnt = np.bincount(sw, minlength=W2)
    start = np.concatenate([[0], np.cumsum(cnt)[:-1]])
    within = np.arange(len(order)) - start[sw]
    assert within.max(initial=0) < K, f"L2 cell overflow {within.max()}"
    slot = sw * K + within

    nslots = W2 * K
    gidx = np.zeros(nslots, np.int64)
    gwv = np.zeros(nslots, np.float32)
    rk = np.full(nslots, -1000.0, np.float32)
    gidx[slot] = gsrc_local[order]
    gwv[slot] = we[order]
    rk[slot] = r[order]

    ncalls = cfg.CALLS2
    per = cfg.NB2 * K
    return {
        "gidx2": np.concatenate(
            [_wrap_idx(gidx.reshape(ncalls, per)[i]) for i in range(ncalls)], axis=1),
        "dra2": np.concatenate(
            [_wrap_col(rk.reshape(ncalls, per)[i], np.float32) for i in range(ncalls)], axis=1),
        "gw2": np.concatenate(
            [_wrap_col(gwv.reshape(ncalls, per)[i], np.float32) for i in range(ncalls)], axis=1),
    }


def prep(x, W1, b1, W2, b2, edge_index, cfg):
    N, R, NC = cfg.N, cfg.R, cfg.NC
    src = np.asarray(edge_index[0], np.int64)
    dst = np.asarray(edge_index[1], np.int64)
    x_bf = np.ascontiguousarray(np.asarray(x, np.float32).astype(NPBF16))
    deg = np.ones(N, np.float32)
    np.add.at(deg, dst, 1.0)
    dinv = (1.0 / np.sqrt(deg)).astype(np.float32)

    # ---- L1: per-dst-owner edges
    owner_d = dst // R
    per_core_l1 = []
    for k in range(NC):
        m = owner_d == k
        per_core_l1.append((src[m], dst[m] - k * R))

    pk1 = []
    for k in range(NC):
        s_k, d_k = per_core_l1[k]
        cnt = np.zeros((R, 4), np.int64)
        np.add.at(cnt, (d_k, s_k // cfg.CH1), 1)
        r = _pack4(cnt, cfg.NWIN1, cfg.K1)
        assert r is not None, f"L1 packing failed core {k}"
        pk1.append(r)

    # local compact position of every node (within its owner's h2s_c,
    # r-major layout: row = rank * NWIN1 + window)
    cpos_local = np.empty(N, np.int64)
    for k in range(NC):
        a1, r1 = pk1[k]
        cpos_local[k * R:(k + 1) * R] = r1 * cfg.NWIN1 + a1

    # ---- L2: global window packing on per-src-owner dst counts (+ self)
    owner_s = src // R
    cnt8 = np.zeros((N, NC), np.int32)
    np.add.at(cnt8, (dst, owner_s), 1)
    cnt8[np.arange(N), np.arange(N) // R] += 1
    pk2 = None
    for W2G in (cfg.W2, cfg.W2 + 16, cfg.W2 + 32, cfg.W2 + 64):
        if W2G != cfg.W2:
            cfg = Cfg(W2=W2G)
        pk2 = _pack8(cnt8, cfg.W2, cfg.K2)
        if pk2 is not None:
            break
    assert pk2 is not None, "L2 global packing failed"
    a2, r2 = pk2

    iota = np.tile(np.arange(128, dtype=np.float32), (128, 1)).astype(NPBF16)
    b1w = np.zeros((128, cfg.HID // 128), np.float32)
    for h in range(cfg.HID):
        b1w[h % 128, h // 128] = b1[h]
    b2bc = np.tile(np.asarray(b2, np.float32), (128, 8)).astype(NPBF16)

    in_maps = []
    for k in range(NC):
        s_k, d_k = per_core_l1[k]
        a1, r1 = pk1[k]
        dloc = dinv[k * R:(k + 1) * R]

        st1 = _emit_l1(cfg, d_k, s_k, dinv[s_k] * dloc[d_k], dloc * dloc, a1, r1)

        # L2 edges: src owned by k (+ self loops of k's nodes)
        m = owner_s == k
        l2_dst = np.concatenate([dst[m], np.arange(k * R, (k + 1) * R)])
        l2_srcl = np.concatenate([cpos_local[src[m]], cpos_local[k * R:(k + 1) * R]])
        l2_we = dinv[np.concatenate([dst[m], np.arange(k * R, (k + 1) * R)])]
        st2 = _emit_l2(cfg, l2_dst, l2_srcl, l2_we, a2, r2)

        dc = np.zeros((cfg.NWIN1, 128), np.float32)
        dc[a1, r1] = dloc

        in_maps.append({
            "x": x_bf,
            "x_own": np.ascontiguousarray(x_bf[k * R:(k + 1) * R]),
            "W1": np.asarray(W1, np.float32),
            "W2w": np.asarray(W2, np.float32),
            "b1w": b1w,
            "b2bc": b2bc,
            "iota": iota,
            "dinv1": np.ascontiguousarray(dc.T),
            **st1,
            **st2,
        })

    # host map: node -> (owner core, outc partition p, outc column wcol)
    # partial_j is r-major [128, Wj, C] over its window subrange; RS chunk j
    # scatters flat (rank, window) rows; core k gets ranks [16k, 16k+16).
    wsplit = np.asarray(cfg.WSPLIT)
    wstart = np.concatenate([[0], np.cumsum(wsplit)[:-1]])
    fstart = np.concatenate([[0], np.cumsum(cfg.FWINS)[:-1]])
    j = np.searchsorted(np.cumsum(wsplit), a2, side="right")
    wj = a2 - wstart[j]
    RPC = 128 // cfg.NC
    owner = r2 // RPC
    i = (r2 % RPC) * wsplit[j] + wj
    p = i % 128
    wcol = fstart[j] + i // 128
    return cfg, in_maps, owner, p, wcol


# ----------------------------------------------------------------------------
# device program
# ----------------------------------------------------------------------------

def build_program(cfg):
    nc = bacc.Bacc(None, target_bir_lowering=False, debug=False)
    F, HID, C = cfg.F, cfg.HID, cfg.C
    NB1, NCOL1, NBATCH1 = cfg.NB1, cfg.NCOL1, cfg.NBATCH1

    x = nc.declare_dram_parameter("x", [cfg.N, F], BF16, isOutput=False)
    x_own = nc.declare_dram_parameter("x_own", [cfg.R, F], BF16, isOutput=False)
    W1p = nc.declare_dram_parameter("W1", [F, HID], F32, isOutput=False)
    W2p = nc.declare_dram_parameter("W2w", [HID, C], F32, isOutput=False)
    b1w = nc.declare_dram_parameter("b1w", [128, HID // 128], F32, isOutput=False)
    b2bc = nc.declare_dram_parameter("b2bc", [128, 8 * C], BF16, isOutput=False)
    iota = nc.declare_dram_parameter("iota", [128, 128], BF16, isOutput=False)
    dinv1 = nc.declare_dram_parameter("dinv1", [128, cfg.NWIN1], F32, isOutput=False)
    gidx1 = nc.declare_dram_parameter("gidx1", [128, NBATCH1 * cfg.GIDX1_B], I16, isOutput=False)
    dra1 = nc.declare_dram_parameter("dra1", [128, NBATCH1 * cfg.DRA1_B], F32, isOutput=False)
    drb1 = nc.declare_dram_parameter("drb1", [128, NBATCH1 * cfg.DRB1_B], F32, isOutput=False)
    gw1 = nc.declare_dram_parameter("gw1", [128, NBATCH1 * cfg.DRA1_B], F32, isOutput=False)
    gidx2 = nc.declare_dram_parameter("gidx2", [128, cfg.CALLS2 * 64], I16, isOutput=False)
    dra2 = nc.declare_dram_parameter("dra2", [128, cfg.W2], F32, isOutput=False)
    gw2 = nc.declare_dram_parameter("gw2", [128, cfg.W2], F32, isOutput=False)
    outc = nc.declare_dram_parameter(
        "outc", [128, cfg.FWIN_TOT, C], F32, isOutput=True)

    # r-major 3D layouts so staging DMAs are contiguous per partition
    h2s_c = nc.dram_tensor("h2s_c", [128, cfg.NWIN1, F], BF16)
    partials = [nc.dram_tensor(f"partial{j}", [128, w, C], BF16)
                for j, w in enumerate(cfg.WSPLIT)]
    rs_out = [nc.dram_tensor(f"rs_out{j}", [fw * 128, C], BF16)
              for j, fw in enumerate(cfg.FWINS)]

    with tile.TileContext(nc) as tc:
        with (
            tc.tile_pool(name="const", bufs=1) as pc,
            tc.tile_pool(name="gpool", bufs=3) as pg,
            tc.tile_pool(name="spool", bufs=2) as ps,
            tc.tile_pool(name="dense", bufs=2) as pd,
            tc.tile_pool(name="psagg", bufs=2, space="PSUM") as ppa,
            tc.tile_pool(name="psdense", bufs=2, space="PSUM") as ppd,
            tc.tile_pool(name="psl2", bufs=2, space="PSUM") as ppl,
        ):
            def load(param, shape, dtype, tag):
                t = pc.tile(shape, dtype, tag=tag)
                nc.sync.dma_start(out=t[:], in_=param[:, :])
                return t

            # L1-critical streams first so the first gathers start ASAP
            streams = {}
            for nm, par, wid, dt in (
                ("gidx1", gidx1, NBATCH1 * cfg.GIDX1_B, I16),
                ("dra1", dra1, NBATCH1 * cfg.DRA1_B, F32),
                ("gw1", gw1, NBATCH1 * cfg.DRA1_B, F32),
            ):
                streams[nm] = load(par, [128, wid], dt, "st_" + nm)
            iota_sb = load(iota, [128, 128], BF16, "iotasb")
            W1f = load(W1p, [F, HID], F32, "W1f")
            W2fa = pc.tile([128, C], F32, tag="W2fa")
            W2fb = pc.tile([128, C], F32, tag="W2fb")
            nc.sync.dma_start(out=W2fa[:], in_=W2p[0:128, :])
            nc.sync.dma_start(out=W2fb[:], in_=W2p[128:256, :])
            b1_sb = load(b1w, [128, HID // 128], F32, "b1sb")
            b2_sb = load(b2bc, [128, 8 * C], BF16, "b2sb")
            dinv1_sb = load(dinv1, [128, cfg.NWIN1], F32, "dinv1sb")

            W1_d = pc.tile([F, HID], BF16, tag="W1d")
            nc.vector.tensor_copy(W1_d[:], W1f[:])
            W2a_d = pc.tile([128, C], BF16, tag="W2ad")
            W2b_d = pc.tile([128, C], BF16, tag="W2bd")
            nc.vector.tensor_copy(W2a_d[:], W2fa[:])
            nc.vector.tensor_copy(W2b_d[:], W2fb[:])

            for nm, par, wid, dt in (
                ("drb1", drb1, NBATCH1 * cfg.DRB1_B, F32),
                ("gidx2", gidx2, cfg.CALLS2 * 64, I16),
                ("dra2", dra2, cfg.W2, F32),
                ("gw2", gw2, cfg.W2, F32),
            ):
                streams[nm] = load(par, [128, wid], dt, "st_" + nm)

            # two persistent h2s staging tiles, pad columns zeroed once
            hstages = []
            for i in range(2):
                t = pc.tile([128, NB1, F], BF16, tag=f"hst{i}")
                nc.vector.memset(t[:], 0.0)
                hstages.append(t)

            # pin the one act table containing Relu/Copy/Exp/Ln so no
            # mid-kernel table reloads happen
            warm = pc.tile([128, 1], F32, tag="warm")
            nc.scalar.activation(warm[:], b1_sb[:, 0:1], AF.Ln)
            nc.scalar.activation(warm[:], b1_sb[:, 0:1], AF.Exp)

            # ---------------- L1 ----------------
            gi1, da1, db1, gwt1 = (streams["gidx1"], streams["dra1"],
                                   streams["drb1"], streams["gw1"])
            from concourse.bass import _add_dep_helper as _adh

            def _add_dep_l1(a, b):
                _adh(a.ins, b.ins, sync=True, reason="p2 bank-zero after relu_b")

            def dense_l1(b, bank):
                aggT = pd.tile([128, NB1 * 128], BF16, tag="aggT")
                nc.scalar.activation(aggT[:], bank[:], AF.Copy)
                hstage = hstages[b % 2]
                for wl in range(NB1):
                    w = b * NB1 + wl
                    a_sl = aggT[:, wl * 128:(wl + 1) * 128]
                    # ph and p2 share one PSUM bank tile: p2's start=True
                    # zeroes the whole bank, so p2's first matmul must wait
                    # for BOTH relus (the hT dep covers relu_a; relu_b is
                    # pinned explicitly below)
                    phb = ppd.tile([128, HID + C], F32, tag="ph", bufs=3)
                    ph = phb[:, 0:HID]
                    nc.tensor.matmul(ph[:, 0:128], lhsT=W1_d[:, 0:128],
                                     rhs=a_sl, start=True, stop=False,
                                     skip_group_check=True)
                    nc.tensor.matmul(ph[:, 128:256], lhsT=W1_d[:, 128:256],
                                     rhs=a_sl, start=False, stop=True,
                                     skip_group_check=True)
                    hT = pd.tile([128, HID], BF16, tag="hT")
                    nc.scalar.activation(hT[:, 0:128], ph[:, 0:128],
                                         AF.Relu, bias=b1_sb[:, 0:1])
                    relu_b = nc.scalar.activation(hT[:, 128:256], ph[:, 128:256],
                                                  AF.Relu, bias=b1_sb[:, 1:2])
                    p2 = phb[:, HID:HID + C]
                    mm1 = nc.tensor.matmul(p2, lhsT=hT[:, 0:128], rhs=W2a_d[:],
                                           start=True, stop=False,
                                           skip_group_check=True)
                    _add_dep_l1(mm1, relu_b)
                    nc.tensor.matmul(p2, lhsT=hT[:, 128:256], rhs=W2b_d[:],
                                     start=False, stop=True,
                                     skip_group_check=True)
                    nc.vector.tensor_scalar(hstage[:, wl, 0:C], p2,
                                            dinv1_sb[:, w:w + 1], None, OP.mult)
                nc.sync.dma_start(
                    out=h2s_c[:, b * NB1:(b + 1) * NB1, :], in_=hstage[:])

            for b in range(NBATCH1):
                go = b * cfg.GIDX1_B
                ao = b * cfg.DRA1_B
                bank = ppa.tile([128, NB1 * 128], F32, tag="aggps")

                first = True
                for c in range(4):
                    G = pg.tile([128, NCOL1, F // 2], U32, tag="G1")
                    nc.gpsimd.dma_gather(
                        G[:], x[c * cfg.CH1:(c + 1) * cfg.CH1, :].bitcast(U32),
                        gi1[:, go + c * (cfg.IDX_REAL // 16):
                            go + (c + 1) * (cfg.IDX_REAL // 16)],
                        cfg.IDX_REAL, cfg.IDX_REAL, F // 2)
                    Gbf = G[:].bitcast(BF16)
                    for j, (wa, split) in enumerate(cfg.colmap):
                        for wl, second in ((wa, False), (wa + 1, True)) if split else ((wa, False),):
                            if wl >= NB1:
                                continue
                            col = ao + c * NCOL1 + j
                            S = ps.tile([128, 128], BF16, tag="S", bufs=6)
                            dsrc = db1 if second else da1
                            dcol = (b * cfg.DRB1_B + c * NCOL1 + j) if second else col
                            eng = nc.gpsimd if c == 3 else nc.vector
                            eng.tensor_scalar(
                                S[:], iota_sb[:],
                                dsrc[:, dcol:dcol + 1], gwt1[:, col:col + 1],
                                OP.is_equal, OP.mult)
                            nc.tensor.matmul(
                                bank[:, wl * 128:(wl + 1) * 128],
                                lhsT=Gbf[:, j, :], rhs=S[:],
                                start=first, stop=False, skip_group_check=True)
                            first = False
                # self
                Gs = pg.tile([128, NB1, F // 2], U32, tag="Gs1")
                so = go + 4 * (cfg.IDX_REAL // 16)
                nc.gpsimd.dma_gather(
                    Gs[:], x_own[:, :].bitcast(U32),
                    gi1[:, so:so + cfg.IDX_SELF // 16],
                    cfg.IDX_SELF, cfg.IDX_SELF, F // 2)
                Gsbf = Gs[:].bitcast(BF16)
                for wl in range(NB1):
                    col = ao + 4 * NCOL1 + wl
                    S = ps.tile([128, 128], BF16, tag="S", bufs=6)
                    nc.gpsimd.tensor_scalar(
                        S[:], iota_sb[:],
                        da1[:, col:col + 1], gwt1[:, col:col + 1],
                        OP.is_equal, OP.mult)
                    nc.tensor.matmul(
                        bank[:, wl * 128:(wl + 1) * 128],
                        lhsT=Gsbf[:, wl, :], rhs=S[:],
                        start=False, stop=(wl == NB1 - 1),
                        skip_group_check=True)
                dense_l1(b, bank)

            # ---------------- L2: src-sharded partials ----------------
            gi2, da2, gwt2 = streams["gidx2"], streams["dra2"], streams["gw2"]
            NB2, K2 = cfg.NB2, cfg.K2
            call_split = [w // NB2 for w in cfg.WSPLIT]

            last_gather = None
            for call in range(cfg.CALLS2):
                G2 = pg.tile([128, NB2, F // 2], U32, tag="G2", bufs=24)
                last_gather = nc.gpsimd.dma_gather(
                    G2[:],
                    h2s_c[:, :, :].rearrange("r w f -> (r w) f").bitcast(U32),
                    gi2[:, call * 64:(call + 1) * 64],
                    NB2 * K2, NB2 * K2, F // 2)
                G2bf = G2[:].bitcast(BF16)
                bank2 = ppl.tile([128, NB2 * C], F32, tag="l2ps", bufs=3)
                for wl in range(NB2):
                    w = call * NB2 + wl
                    S = ps.tile([128, 128], BF16, tag="S2", bufs=12)
                    eng = nc.vector
                    si = eng.tensor_scalar(
                        S[:], iota_sb[:],
                        da2[:, w:w + 1], gwt2[:, w:w + 1],
                        OP.is_equal, OP.mult)
                    nc.tensor.matmul(
                        bank2[:, wl * C:(wl + 1) * C],
                        lhsT=S[:], rhs=G2bf[:, wl, 0:C],
                        start=(wl == 0), stop=(wl == NB2 - 1),
                        skip_group_check=True)
                stage = pd.tile([128, NB2, C], BF16, tag="pstage", bufs=4)
                nc.scalar.activation(stage[:], bank2[:], AF.Copy)
                pj = 0 if call < call_split[0] else 1
                pcall = call - (0 if pj == 0 else call_split[0])
                last_pdma = nc.sync.dma_start(
                    out=partials[pj][:, pcall * NB2:(pcall + 1) * NB2, :],
                    in_=stage[:])

            # Collectives and the rs_out loads all live on the Pool queue,
            # explicitly chained (RS0 -> lt0 -> RS1 -> lt1) and pinned after
            # the last gather: the tile scheduler otherwise hoists the
            # collectives and stalls the L2 pipeline behind them.
            from concourse.bass import _add_dep_helper
            lts = []
            prev = last_gather
            dprev = last_pdma
            for j in range(cfg.NRS):
                cc = nc.gpsimd.collective_compute(
                    "ReduceScatter", OP.add,
                    ins=[partials[j][:, :, :]],
                    outs=[rs_out[j][:, :]],
                    replica_groups=[list(range(cfg.NC))],
                )
                _add_dep_helper(cc.ins, prev.ins, sync=True,
                                reason="keep Pool queue order")
                prev = cc
                FW = cfg.FWINS[j]
                lt = pd.tile([128, FW, C], BF16, tag=f"ltall{j}", bufs=1)
                ld = nc.sync.dma_start(
                    out=lt[:],
                    in_=rs_out[j][:, :].rearrange("(w r) c -> r w c", w=FW))
                _add_dep_helper(ld.ins, dprev.ins, sync=True,
                                reason="keep SP queue order")
                lts.append(lt)
                dprev = ld

            # ---------------- final: +b2, log_softmax on received rows ----
            # (all on DVE/ACT/SP: Pool is busy with the collectives)
            fcol = 0
            for j in range(cfg.NRS):
                FW = cfg.FWINS[j]
                lt = lts[j]
                # logits are O(+-40) so exp cannot overflow f32: skip the
                # usual max-shift and compute log_softmax = x - ln(sum(exp x)),
                # batching 8 windows per op
                xs_all = pc.tile([128, FW * C], F32, tag=f"xs{j}")
                ss_all = pc.tile([128, FW], F32, tag=f"ss{j}")
                for g0 in range(0, FW, 8):
                    gn = min(8, FW - g0)
                    xs = xs_all[:, g0 * C:(g0 + gn) * C]
                    nc.vector.tensor_add(
                        xs, lt[:, g0:g0 + gn, :], b2_sb[:, 0:gn * C])
                    es = pd.tile([128, 8 * C], F32, tag="es")
                    nc.scalar.activation(es[:, 0:gn * C], xs, AF.Exp)
                    nc.vector.tensor_reduce(
                        ss_all[:, g0:g0 + gn],
                        es[:, 0:gn * C].rearrange("p (g c) -> p g c", g=gn),
                        mybir.AxisListType.X, OP.add)
                ls_all = pc.tile([128, FW], F32, tag=f"ls{j}")
                nc.scalar.activation(ls_all[:], ss_all[:], AF.Ln)
                ost = pd.tile([128, FW, C], F32, tag=f"ost{j}", bufs=1)
                for fw in range(FW):
                    nc.vector.tensor_scalar(
                        ost[:, fw, :], xs_all[:, fw * C:(fw + 1) * C],
                        ls_all[:, fw:fw + 1], None, OP.subtract)
                nc.sync.dma_start(
                    out=outc[:, fcol:fcol + FW, :], in_=ost[:])
                fcol += FW

    nc.compile()
    return nc


_PROGRAM_CACHE = {}


def _get_program(cfg):
    key = (cfg.N, cfg.NWIN1, cfg.NB1, cfg.K1, cfg.W2, cfg.NRS)
    if key not in _PROGRAM_CACHE:
        _PROGRAM_CACHE[key] = build_program(cfg)
    return _PROGRAM_CACHE[key]


def kernel(x, W1, b1, W2, b2, edge_index):
    cfg, in_maps, owner, p, wcol = prep(x, W1, b1, W2, b2, edge_index, CFG_FULL)
    nc = _get_program(cfg)
    res = run_bass_kernel_spmd(
        nc, in_maps, core_ids=list(range(cfg.NC)),
        trace=bool(os.environ.get("GCN_TRACE")))
    if res.exec_time_ns is not None:
        print(f"HW exec time: {res.exec_time_ns} ns")
    outs = np.stack([res.results[k]["outc"] for k in range(cfg.NC)])
    return outs[owner, p, wcol].astype(np.float32)


# revision 71
# speedup vs baseline: 2.6110x; 1.0341x over previous
"""GCN (2-layer, symmetric-normalized A+I) on 8 Trainium2 NeuronCores.

Strategy:
  L1 (dst-sharded, no comm): x is replicated, so core k aggregates x[src] for
  its own 12500 dst nodes via one-hot matmul segment-sum (bin-packed windows
  of 128 nodes x 4 src-chunk cells), then dense W1/relu/W2, scaling rows by
  dinv[d] into h2s_c (local compact order, rows padded to 256B for gather).

  L2 (src-sharded partials + ReduceScatter): core k owns h2 for its own nodes
  only.  It processes exactly the edges whose SRC it owns (plus self-loops of
  its own nodes), gathering from LOCAL h2s_c, and accumulates weighted
  one-hot matmul partial sums over a GLOBAL window layout shared by all cores
  (host packs all N nodes into W2 windows of <=128 nodes with per-core edge
  load <= 128).  The [W2*128, 64] bf16 partial buffers are then summed across
  cores with two chunked ReduceScatter(+add) collectives (output = 1/8th of
  the data => cheap), and each core finishes log_softmax on the compact rows
  it receives.  The host maps compact rows back to natural node order.

  Gathers are declared uint32 (256B rows = 64 x u32) and bitcast back to bf16
  for the PE, which halves the gather element count.

kernel(**inputs) takes FULL inputs and returns the FULL [100000, 64] output.
"""
import os
import numpy as np

os.environ.setdefault("NEURON_SCRATCHPAD_PAGE_SIZE", "256")

import concourse.bacc as bacc
import concourse.tile as tile
import concourse.mybir as mybir
from concourse import bass
from concourse.bass_utils import run_bass_kernel_spmd

F32 = mybir.dt.float32
BF16 = mybir.dt.bfloat16
I16 = mybir.dt.int16
U32 = mybir.dt.uint32
U64 = mybir.dt.uint64
NPBF16 = mybir.dt.np(BF16)

AF = mybir.ActivationFunctionType
OP = mybir.AluOpType


class Cfg:
    def __init__(self, N=100000, R=12500, F=128, HID=256, C=64, NC=8,
                 NWIN1=100, NB1=4, K1=256, W2=784, NRS=2):
        self.N, self.R, self.F, self.HID, self.C, self.NC = N, R, F, HID, C, NC
        self.NWIN1, self.NB1, self.K1 = NWIN1, NB1, K1
        self.NBATCH1 = NWIN1 // NB1
        assert NB1 * K1 <= 1024 and (NB1 * K1) % 128 == 0
        self.NCOL1 = NB1 * K1 // 128
        self.IDX_REAL = NB1 * K1
        self.IDX_SELF = NB1 * 128
        self.CH1 = N // 4
        self.CROWS = NWIN1 * 128          # h2s rows per core
        assert self.CH1 <= 32768 and self.CROWS <= 32768
        # L1 column->cell map (no splits for K1=256)
        self.colmap = []
        for j in range(self.NCOL1):
            a = (j * 128) // K1
            self.colmap.append((a, (j * 128 + 127) // K1 != a))
        self.GIDX1_B = 4 * self.IDX_REAL // 16 + self.IDX_SELF // 16
        self.DRA1_B = 4 * self.NCOL1 + NB1
        self.DRB1_B = 4 * self.NCOL1
        # L2
        self.W2 = W2
        self.NB2 = 8
        self.K2 = 128
        assert W2 % (self.NB2 * NRS) == 0 and W2 % 16 == 0
        self.CALLS2 = W2 // self.NB2
        self.NRS = NRS
        # window split across RS chunks (large first, small last to shrink
        # the post-L2 tail); each chunk gets its own partial tensor
        big = (W2 * 7 // 10 + 7) // 8 * 8
        self.WSPLIT = [big, W2 - big] if NRS == 2 else [W2]
        assert all(w % self.NB2 == 0 for w in self.WSPLIT)
        self.FWINS = [w * 128 // NC // 128 for w in self.WSPLIT]
        assert all(w * 128 % (NC * 128) == 0 for w in self.WSPLIT)
        self.FWIN_TOT = sum(self.FWINS)


CFG_FULL = Cfg()


# ----------------------------------------------------------------------------
# host prep
# ----------------------------------------------------------------------------

def _pack4(cnt, nwin, K, WN=128):
    """Bin-pack nodes (rows of cnt [n,4]) into nwin windows, per-cell cap K,
    count cap WN. Returns (assign, rank) or None."""
    degs = cnt.sum(1)
    order = np.argsort(-degs, kind="stable")
    loads = np.zeros((nwin, 4), np.int64)
    counts = np.zeros(nwin, np.int64)
    assign = np.full(len(degs), -1, np.int64)
    rank = np.full(len(degs), -1, np.int64)
    for i in order:
        c = cnt[i]
        ok = (counts < WN) & ((loads + c) <= K).all(axis=1)
        if not ok.any():
            return None
        score = (loads + c).max(axis=1) * 1000 + counts
        score = np.where(ok, score, 1 << 40)
        w = int(np.argmin(score))
        assign[i] = w
        rank[i] = counts[w]
        counts[w] += 1
        loads[w] += c
    return assign, rank


def _pack8(cnt8, W, K=128, WN=128):
    """Global 8-dim pack: all N nodes into W windows; per-core load cap K,
    count cap WN. Returns (assign, rank) or None."""
    n = len(cnt8)
    tot = cnt8.sum(1)
    order = np.argsort(-tot, kind="stable")
    loads = np.zeros((W, 8), np.int32)
    counts = np.zeros(W, np.int32)
    assign = np.full(n, -1, np.int64)
    rank = np.full(n, -1, np.int64)
    big = np.int64(1) << 40
    for i in order:
        c = cnt8[i]
        nl = loads + c[None, :]
        ok = (counts < WN) & (nl <= K).all(axis=1)
        if not ok.any():
            return None
        score = nl.max(axis=1).astype(np.int64) * 256 + counts
        score = np.where(ok, score, big)
        w = int(np.argmin(score))
        assign[i] = w
        rank[i] = counts[w]
        counts[w] += 1
        loads[w] = nl[w]
    return assign, rank


def _wrap_idx(a):
    """[..., n] -> [..., 128, n//16]; idx i at [i%16, i//16], replicated x8."""
    lead = a.shape[:-1]
    n = a.shape[-1]
    w = a.reshape(*lead, n // 16, 16)
    w = np.moveaxis(w, -1, -2)
    return np.tile(w, (*([1] * len(lead)), 8, 1)).astype(np.int16)


def _wrap_col(a, dt):
    """[..., n] -> [..., 128, n//128]; slot p at [p%128, p//128]."""
    lead = a.shape[:-1]
    n = a.shape[-1]
    w = a.reshape(*lead, n // 128, 128)
    return np.ascontiguousarray(np.moveaxis(w, -1, -2)).astype(dt)


def _emit_l1(cfg, dl, gsrc, we, self_w, assign, rank):
    """Per-core L1 streams. dl/gsrc/we: local dst, global src, edge weight.
    self gather index = local node id (into x_own); self_w per local node."""
    NB, K, NCOL, NBATCH = cfg.NB1, cfg.K1, cfg.NCOL1, cfg.NBATCH1
    IDX_REAL, IDX_SELF = cfg.IDX_REAL, cfg.IDX_SELF
    CH = cfg.CH1
    chunk = gsrc // CH
    w = assign[dl]
    r = rank[dl]
    key = w * 4 + chunk
    order = np.argsort(key, kind="stable")
    skey = key[order]
    cellcnt = np.bincount(skey, minlength=cfg.NWIN1 * 4)
    cellstart = np.concatenate([[0], np.cumsum(cellcnt)[:-1]])
    within = np.arange(len(order)) - cellstart[skey]
    assert within.max(initial=0) < K, f"L1 cell overflow {within.max()}"
    bb = w[order] // NB
    wl = w[order] % NB
    slot = wl * K + within
    ch = chunk[order]

    gidx = np.zeros((NBATCH, 4, IDX_REAL), np.int64)
    gwv = np.zeros((NBATCH, 4, IDX_REAL), np.float32)
    rk = np.full((NBATCH, 4, IDX_REAL), -1000.0, np.float32)
    gidx[bb, ch, slot] = gsrc[order] - ch * CH
    gwv[bb, ch, slot] = we[order]
    rk[bb, ch, slot] = r[order]

    slots = np.arange(IDX_REAL)
    cell_of = slots // K
    wlA = ((slots // 128) * 128) // K
    dra = np.where(cell_of[None, None, :] == wlA[None, None, :], rk, -1000.0)
    drb = np.where(cell_of[None, None, :] == (wlA + 1)[None, None, :], rk, -1000.0)

    # self call
    node_at = np.full((cfg.NWIN1, 128), -1, np.int64)
    node_at[assign, rank] = np.arange(len(assign))
    sidx = np.zeros((NBATCH, IDX_SELF), np.int64)
    sw = np.zeros((NBATCH, IDX_SELF), np.float32)
    sdr = np.full((NBATCH, IDX_SELF), -1000.0, np.float32)
    rr = np.arange(IDX_SELF) % 128
    for b in range(NBATCH):
        flat = node_at[b * NB:(b + 1) * NB].reshape(-1)
        valid = flat >= 0
        sidx[b][valid] = flat[valid]
        sw[b][valid] = self_w[flat[valid]]
        sdr[b][valid] = rr[valid]

    gidx_parts, dra_parts, drb_parts, gw_parts = [], [], [], []
    for b in range(NBATCH):
        for c in range(4):
            gidx_parts.append(_wrap_idx(gidx[b, c]))
            dra_parts.append(_wrap_col(dra[b, c], np.float32))
            drb_parts.append(_wrap_col(drb[b, c], np.float32))
            gw_parts.append(_wrap_col(gwv[b, c], np.float32))
        gidx_parts.append(_wrap_idx(sidx[b]))
        dra_parts.append(_wrap_col(sdr[b], np.float32))
        gw_parts.append(_wrap_col(sw[b], np.float32))
    return {
        "gidx1": np.concatenate(gidx_parts, axis=1),
        "dra1": np.concatenate(dra_parts, axis=1),
        "drb1": np.concatenate(drb_parts, axis=1),
        "gw1": np.concatenate(gw_parts, axis=1),
    }


def _emit_l2(cfg, dst_g, gsrc_local, we, assign2, rank2):
    """Per-core L2 streams over the GLOBAL window layout."""
    W2, K = cfg.W2, cfg.K2
    w = assign2[dst_g]
    r = rank2[dst_g]
    order = np.argsort(w, kind="stable")
    sw = w[order]
    cnt = np.bincount(sw, minlength=W2)
    start = np.concatenate([[0], np.cumsum(cnt)[:-1]])
    within = np.arange(len(order)) - start[sw]
    assert within.max(initial=0) < K, f"L2 cell overflow {within.max()}"
    slot = sw * K + within

    nslots = W2 * K
    gidx = np.zeros(nslots, np.int64)
    gwv = np.zeros(nslots, np.float32)
    rk = np.full(nslots, -1000.0, np.float32)
    gidx[slot] = gsrc_local[order]
    gwv[slot] = we[order]
    rk[slot] = r[order]

    ncalls = cfg.CALLS2
    per = cfg.NB2 * K
    return {
        "gidx2": np.concatenate(
            [_wrap_idx(gidx.reshape(ncalls, per)[i]) for i in range(ncalls)], axis=1),
        "dra2": np.concatenate(
            [_wrap_col(rk.reshape(ncalls, per)[i], np.float32) for i in range(ncalls)], axis=1),
        "gw2": np.concatenate(
            [_wrap_col(gwv.reshape(ncalls, per)[i], np.float32) for i in range(ncalls)], axis=1),
    }


def prep(x, W1, b1, W2, b2, edge_index, cfg):
    N, R, NC = cfg.N, cfg.R, cfg.NC
    src = np.asarray(edge_index[0], np.int64)
    dst = np.asarray(edge_index[1], np.int64)
    x_bf = np.ascontiguousarray(np.asarray(x, np.float32).astype(NPBF16))
    deg = np.ones(N, np.float32)
    np.add.at(deg, dst, 1.0)
    dinv = (1.0 / np.sqrt(deg)).astype(np.float32)

    # ---- L1: per-dst-owner edges
    owner_d = dst // R
    per_core_l1 = []
    for k in range(NC):
        m = owner_d == k
        per_core_l1.append((src[m], dst[m] - k * R))

    pk1 = []
    for k in range(NC):
        s_k, d_k = per_core_l1[k]
        cnt = np.zeros((R, 4), np.int64)
        np.add.at(cnt, (d_k, s_k // cfg.CH1), 1)
        r = _pack4(cnt, cfg.NWIN1, cfg.K1)
        assert r is not None, f"L1 packing failed core {k}"
        pk1.append(r)

    # local compact position of every node (within its owner's h2s_c,
    # r-major layout: row = rank * NWIN1 + window)
    cpos_local = np.empty(N, np.int64)
    for k in range(NC):
        a1, r1 = pk1[k]
        cpos_local[k * R:(k + 1) * R] = r1 * cfg.NWIN1 + a1

    # ---- L2: global window packing on per-src-owner dst counts (+ self)
    owner_s = src // R
    cnt8 = np.zeros((N, NC), np.int32)
    np.add.at(cnt8, (dst, owner_s), 1)
    cnt8[np.arange(N), np.arange(N) // R] += 1
    pk2 = None
    for W2G in (cfg.W2, cfg.W2 + 16, cfg.W2 + 32, cfg.W2 + 64):
        if W2G != cfg.W2:
            cfg = Cfg(W2=W2G)
        pk2 = _pack8(cnt8, cfg.W2, cfg.K2)
        if pk2 is not None:
            break
    assert pk2 is not None, "L2 global packing failed"
    a2, r2 = pk2

    iota = np.tile(np.arange(128, dtype=np.float32), (128, 1)).astype(NPBF16)
    b1w = np.zeros((128, cfg.HID // 128), np.float32)
    for h in range(cfg.HID):
        b1w[h % 128, h // 128] = b1[h]
    b2bc = np.tile(np.asarray(b2, np.float32), (128, 8)).astype(NPBF16)

    in_maps = []
    for k in range(NC):
        s_k, d_k = per_core_l1[k]
        a1, r1 = pk1[k]
        dloc = dinv[k * R:(k + 1) * R]

        st1 = _emit_l1(cfg, d_k, s_k, dinv[s_k] * dloc[d_k], dloc * dloc, a1, r1)

        # L2 edges: src owned by k (+ self loops of k's nodes)
        m = owner_s == k
        l2_dst = np.concatenate([dst[m], np.arange(k * R, (k + 1) * R)])
        l2_srcl = np.concatenate([cpos_local[src[m]], cpos_local[k * R:(k + 1) * R]])
        l2_we = dinv[np.concatenate([dst[m], np.arange(k * R, (k + 1) * R)])]
        st2 = _emit_l2(cfg, l2_dst, l2_srcl, l2_we, a2, r2)

        dc = np.zeros((cfg.NWIN1, 128), np.float32)
        dc[a1, r1] = dloc

        in_maps.append({
            "x": x_bf,
            "x_own": np.ascontiguousarray(x_bf[k * R:(k + 1) * R]),
            "W1": np.asarray(W1, np.float32),
            "W2w": np.asarray(W2, np.float32),
            "b1w": b1w,
            "b2bc": b2bc,
            "iota": iota,
            "dinv1": np.ascontiguousarray(dc.T),
            **st1,
            **st2,
        })

    # host map: node -> (owner core, outc partition p, outc column wcol)
    # partial_j is r-major [128, Wj, C] over its window subrange; RS chunk j
    # scatters flat (rank, window) rows; core k gets ranks [16k, 16k+16).
    wsplit = np.asarray(cfg.WSPLIT)
    wstart = np.concatenate([[0], np.cumsum(wsplit)[:-1]])
    fstart = np.concatenate([[0], np.cumsum(cfg.FWINS)[:-1]])
    j = np.searchsorted(np.cumsum(wsplit), a2, side="right")
    wj = a2 - wstart[j]
    RPC = 128 // cfg.NC
    owner = r2 // RPC
    i = (r2 % RPC) * wsplit[j] + wj
    p = i % 128
    wcol = fstart[j] + i // 128
    return cfg, in_maps, owner, p, wcol


# ----------------------------------------------------------------------------
# device program
# ----------------------------------------------------------------------------

def build_program(cfg):
    nc = bacc.Bacc(None, target_bir_lowering=False, debug=False)
    F, HID, C = cfg.F, cfg.HID, cfg.C
    NB1, NCOL1, NBATCH1 = cfg.NB1, cfg.NCOL1, cfg.NBATCH1

    x = nc.declare_dram_parameter("x", [cfg.N, F], BF16, isOutput=False)
    x_own = nc.declare_dram_parameter("x_own", [cfg.R, F], BF16, isOutput=False)
    W1p = nc.declare_dram_parameter("W1", [F, HID], F32, isOutput=False)
    W2p = nc.declare_dram_parameter("W2w", [HID, C], F32, isOutput=False)
    b1w = nc.declare_dram_parameter("b1w", [128, HID // 128], F32, isOutput=False)
    b2bc = nc.declare_dram_parameter("b2bc", [128, 8 * C], BF16, isOutput=False)
    iota = nc.declare_dram_parameter("iota", [128, 128], BF16, isOutput=False)
    dinv1 = nc.declare_dram_parameter("dinv1", [128, cfg.NWIN1], F32, isOutput=False)
    gidx1 = nc.declare_dram_parameter("gidx1", [128, NBATCH1 * cfg.GIDX1_B], I16, isOutput=False)
    dra1 = nc.declare_dram_parameter("dra1", [128, NBATCH1 * cfg.DRA1_B], F32, isOutput=False)
    drb1 = nc.declare_dram_parameter("drb1", [128, NBATCH1 * cfg.DRB1_B], F32, isOutput=False)
    gw1 = nc.declare_dram_parameter("gw1", [128, NBATCH1 * cfg.DRA1_B], F32, isOutput=False)
    gidx2 = nc.declare_dram_parameter("gidx2", [128, cfg.CALLS2 * 64], I16, isOutput=False)
    dra2 = nc.declare_dram_parameter("dra2", [128, cfg.W2], F32, isOutput=False)
    gw2 = nc.declare_dram_parameter("gw2", [128, cfg.W2], F32, isOutput=False)
    outc = nc.declare_dram_parameter(
        "outc", [128, cfg.FWIN_TOT, C], F32, isOutput=True)

    # r-major 3D layouts so staging DMAs are contiguous per partition
    h2s_c = nc.dram_tensor("h2s_c", [128, cfg.NWIN1, F], BF16)
    partials = [nc.dram_tensor(f"partial{j}", [128, w, C], BF16)
                for j, w in enumerate(cfg.WSPLIT)]
    rs_out = [nc.dram_tensor(f"rs_out{j}", [fw * 128, C], BF16)
              for j, fw in enumerate(cfg.FWINS)]

    with tile.TileContext(nc) as tc:
        with (
            tc.tile_pool(name="const", bufs=1) as pc,
            tc.tile_pool(name="gpool", bufs=3) as pg,
            tc.tile_pool(name="spool", bufs=2) as ps,
            tc.tile_pool(name="dense", bufs=2) as pd,
            tc.tile_pool(name="psagg", bufs=2, space="PSUM") as ppa,
            tc.tile_pool(name="psdense", bufs=2, space="PSUM") as ppd,
            tc.tile_pool(name="psl2", bufs=2, space="PSUM") as ppl,
        ):
            def load(param, shape, dtype, tag):
                t = pc.tile(shape, dtype, tag=tag)
                nc.sync.dma_start(out=t[:], in_=param[:, :])
                return t

            # L1-critical streams first so the first gathers start ASAP
            streams = {}
            for nm, par, wid, dt in (
                ("gidx1", gidx1, NBATCH1 * cfg.GIDX1_B, I16),
                ("dra1", dra1, NBATCH1 * cfg.DRA1_B, F32),
                ("gw1", gw1, NBATCH1 * cfg.DRA1_B, F32),
            ):
                streams[nm] = load(par, [128, wid], dt, "st_" + nm)
            iota_sb = load(iota, [128, 128], BF16, "iotasb")
            W1f = load(W1p, [F, HID], F32, "W1f")
            W2fa = pc.tile([128, C], F32, tag="W2fa")
            W2fb = pc.tile([128, C], F32, tag="W2fb")
            nc.sync.dma_start(out=W2fa[:], in_=W2p[0:128, :])
            nc.sync.dma_start(out=W2fb[:], in_=W2p[128:256, :])
            b1_sb = load(b1w, [128, HID // 128], F32, "b1sb")
            b2_sb = load(b2bc, [128, 8 * C], BF16, "b2sb")
            dinv1_sb = load(dinv1, [128, cfg.NWIN1], F32, "dinv1sb")

            W1_d = pc.tile([F, HID], BF16, tag="W1d")
            nc.vector.tensor_copy(W1_d[:], W1f[:])
            W2a_d = pc.tile([128, C], BF16, tag="W2ad")
            W2b_d = pc.tile([128, C], BF16, tag="W2bd")
            nc.vector.tensor_copy(W2a_d[:], W2fa[:])
            nc.vector.tensor_copy(W2b_d[:], W2fb[:])

            for nm, par, wid, dt in (
                ("drb1", drb1, NBATCH1 * cfg.DRB1_B, F32),
                ("gidx2", gidx2, cfg.CALLS2 * 64, I16),
                ("dra2", dra2, cfg.W2, F32),
                ("gw2", gw2, cfg.W2, F32),
            ):
                streams[nm] = load(par, [128, wid], dt, "st_" + nm)

            # two persistent h2s staging tiles, pad columns zeroed once
            hstages = []
            for i in range(2):
                t = pc.tile([128, NB1, F], BF16, tag=f"hst{i}")
                nc.vector.memset(t[:], 0.0)
                hstages.append(t)

            # pin the one act table containing Relu/Copy/Exp/Ln so no
            # mid-kernel table reloads happen
            warm = pc.tile([128, 1], F32, tag="warm")
            nc.scalar.activation(warm[:], b1_sb[:, 0:1], AF.Ln)
            nc.scalar.activation(warm[:], b1_sb[:, 0:1], AF.Exp)

            # ---------------- L1 ----------------
            gi1, da1, db1, gwt1 = (streams["gidx1"], streams["dra1"],
                                   streams["drb1"], streams["gw1"])
            from concourse.bass import _add_dep_helper as _adh

            def _add_dep_l1(a, b):
                _adh(a.ins, b.ins, sync=True, reason="p2 bank-zero after relu_b")

            def dense_l1(b, bank):
                aggT = pd.tile([128, NB1 * 128], BF16, tag="aggT", bufs=3)
                nc.scalar.activation(aggT[:], bank[:], AF.Copy)
                hstage = hstages[b % 2]
                for wl in range(NB1):
                    w = b * NB1 + wl
                    a_sl = aggT[:, wl * 128:(wl + 1) * 128]
                    # ph and p2 share one PSUM bank tile: p2's start=True
                    # zeroes the whole bank, so p2's first matmul must wait
                    # for BOTH relus (the hT dep covers relu_a; relu_b is
                    # pinned explicitly below)
                    phb = ppd.tile([128, HID + C], F32, tag="ph", bufs=3)
                    ph = phb[:, 0:HID]
                    nc.tensor.matmul(ph[:, 0:128], lhsT=W1_d[:, 0:128],
                                     rhs=a_sl, start=True, stop=False,
                                     skip_group_check=True)
                    nc.tensor.matmul(ph[:, 128:256], lhsT=W1_d[:, 128:256],
                                     rhs=a_sl, start=False, stop=True,
                                     skip_group_check=True)
                    hT = pd.tile([128, HID], BF16, tag="hT", bufs=4)
                    nc.scalar.activation(hT[:, 0:128], ph[:, 0:128],
                                         AF.Relu, bias=b1_sb[:, 0:1])
                    relu_b = nc.scalar.activation(hT[:, 128:256], ph[:, 128:256],
                                                  AF.Relu, bias=b1_sb[:, 1:2])
                    p2 = phb[:, HID:HID + C]
                    mm1 = nc.tensor.matmul(p2, lhsT=hT[:, 0:128], rhs=W2a_d[:],
                                           start=True, stop=False,
                                           skip_group_check=True)
                    _add_dep_l1(mm1, relu_b)
                    nc.tensor.matmul(p2, lhsT=hT[:, 128:256], rhs=W2b_d[:],
                                     start=False, stop=True,
                                     skip_group_check=True)
                    nc.vector.tensor_scalar(hstage[:, wl, 0:C], p2,
                                            dinv1_sb[:, w:w + 1], None, OP.mult)
                nc.sync.dma_start(
                    out=h2s_c[:, b * NB1:(b + 1) * NB1, :], in_=hstage[:])

            for b in range(NBATCH1):
                go = b * cfg.GIDX1_B
                ao = b * cfg.DRA1_B
                bank = ppa.tile([128, NB1 * 128], F32, tag="aggps")

                first = True
                for c in range(4):
                    G = pg.tile([128, NCOL1, F // 2], U32, tag="G1", bufs=6)
                    nc.gpsimd.dma_gather(
                        G[:], x[c * cfg.CH1:(c + 1) * cfg.CH1, :].bitcast(U32),
                        gi1[:, go + c * (cfg.IDX_REAL // 16):
                            go + (c + 1) * (cfg.IDX_REAL // 16)],
                        cfg.IDX_REAL, cfg.IDX_REAL, F // 2)
                    Gbf = G[:].bitcast(BF16)
                    for j, (wa, split) in enumerate(cfg.colmap):
                        for wl, second in ((wa, False), (wa + 1, True)) if split else ((wa, False),):
                            if wl >= NB1:
                                continue
                            col = ao + c * NCOL1 + j
                            S = ps.tile([128, 128], BF16, tag="S", bufs=10)
                            dsrc = db1 if second else da1
                            dcol = (b * cfg.DRB1_B + c * NCOL1 + j) if second else col
                            eng = nc.gpsimd if c == 3 else nc.vector
                            eng.tensor_scalar(
                                S[:], iota_sb[:],
                                dsrc[:, dcol:dcol + 1], gwt1[:, col:col + 1],
                                OP.is_equal, OP.mult)
                            nc.tensor.matmul(
                                bank[:, wl * 128:(wl + 1) * 128],
                                lhsT=Gbf[:, j, :], rhs=S[:],
                                start=first, stop=False, skip_group_check=True)
                            first = False
                # self
                Gs = pg.tile([128, NB1, F // 2], U32, tag="Gs1", bufs=4)
                so = go + 4 * (cfg.IDX_REAL // 16)
                nc.gpsimd.dma_gather(
                    Gs[:], x_own[:, :].bitcast(U32),
                    gi1[:, so:so + cfg.IDX_SELF // 16],
                    cfg.IDX_SELF, cfg.IDX_SELF, F // 2)
                Gsbf = Gs[:].bitcast(BF16)
                for wl in range(NB1):
                    col = ao + 4 * NCOL1 + wl
                    S = ps.tile([128, 128], BF16, tag="S", bufs=10)
                    nc.gpsimd.tensor_scalar(
                        S[:], iota_sb[:],
                        da1[:, col:col + 1], gwt1[:, col:col + 1],
                        OP.is_equal, OP.mult)
                    nc.tensor.matmul(
                        bank[:, wl * 128:(wl + 1) * 128],
                        lhsT=Gsbf[:, wl, :], rhs=S[:],
                        start=False, stop=(wl == NB1 - 1),
                        skip_group_check=True)
                dense_l1(b, bank)

            # ---------------- L2: src-sharded partials ----------------
            gi2, da2, gwt2 = streams["gidx2"], streams["dra2"], streams["gw2"]
            NB2, K2 = cfg.NB2, cfg.K2
            call_split = [w // NB2 for w in cfg.WSPLIT]

            last_gather = None
            for call in range(cfg.CALLS2):
                G2 = pg.tile([128, NB2, F // 2], U32, tag="G2", bufs=24)
                last_gather = nc.gpsimd.dma_gather(
                    G2[:],
                    h2s_c[:, :, :].rearrange("r w f -> (r w) f").bitcast(U32),
                    gi2[:, call * 64:(call + 1) * 64],
                    NB2 * K2, NB2 * K2, F // 2)
                G2bf = G2[:].bitcast(BF16)
                bank2 = ppl.tile([128, NB2 * C], F32, tag="l2ps", bufs=3)
                for wl in range(NB2):
                    w = call * NB2 + wl
                    S = ps.tile([128, 128], BF16, tag="S2", bufs=16)
                    eng = nc.vector
                    si = eng.tensor_scalar(
                        S[:], iota_sb[:],
                        da2[:, w:w + 1], gwt2[:, w:w + 1],
                        OP.is_equal, OP.mult)
                    nc.tensor.matmul(
                        bank2[:, wl * C:(wl + 1) * C],
                        lhsT=S[:], rhs=G2bf[:, wl, 0:C],
                        start=(wl == 0), stop=(wl == NB2 - 1),
                        skip_group_check=True)
                stage = pd.tile([128, NB2, C], BF16, tag="pstage", bufs=6)
                nc.scalar.activation(stage[:], bank2[:], AF.Copy)
                pj = 0 if call < call_split[0] else 1
                pcall = call - (0 if pj == 0 else call_split[0])
                last_pdma = nc.sync.dma_start(
                    out=partials[pj][:, pcall * NB2:(pcall + 1) * NB2, :],
                    in_=stage[:])

            # Collectives and the rs_out loads all live on the Pool queue,
            # explicitly chained (RS0 -> lt0 -> RS1 -> lt1) and pinned after
            # the last gather: the tile scheduler otherwise hoists the
            # collectives and stalls the L2 pipeline behind them.
            from concourse.bass import _add_dep_helper
            lts = []
            prev = last_gather
            dprev = last_pdma
            for j in range(cfg.NRS):
                cc = nc.gpsimd.collective_compute(
                    "ReduceScatter", OP.add,
                    ins=[partials[j][:, :, :]],
                    outs=[rs_out[j][:, :]],
                    replica_groups=[list(range(cfg.NC))],
                )
                _add_dep_helper(cc.ins, prev.ins, sync=True,
                                reason="keep Pool queue order")
                prev = cc
                FW = cfg.FWINS[j]
                lt = pd.tile([128, FW, C], BF16, tag=f"ltall{j}", bufs=1)
                ld = nc.sync.dma_start(
                    out=lt[:],
                    in_=rs_out[j][:, :].rearrange("(w r) c -> r w c", w=FW))
                _add_dep_helper(ld.ins, dprev.ins, sync=True,
                                reason="keep SP queue order")
                lts.append(lt)
                dprev = ld

            # ---------------- final: +b2, log_softmax on received rows ----
            # (all on DVE/ACT/SP: Pool is busy with the collectives)
            fcol = 0
            for j in range(cfg.NRS):
                FW = cfg.FWINS[j]
                lt = lts[j]
                # logits are O(+-40) so exp cannot overflow f32: skip the
                # usual max-shift and compute log_softmax = x - ln(sum(exp x)),
                # batching 8 windows per op
                xs_all = pc.tile([128, FW * C], F32, tag=f"xs{j}")
                ss_all = pc.tile([128, FW], F32, tag=f"ss{j}")
                for g0 in range(0, FW, 8):
                    gn = min(8, FW - g0)
                    xs = xs_all[:, g0 * C:(g0 + gn) * C]
                    nc.vector.tensor_add(
                        xs, lt[:, g0:g0 + gn, :], b2_sb[:, 0:gn * C])
                    es = pd.tile([128, 8 * C], F32, tag="es")
                    nc.scalar.activation(es[:, 0:gn * C], xs, AF.Exp)
                    nc.vector.tensor_reduce(
                        ss_all[:, g0:g0 + gn],
                        es[:, 0:gn * C].rearrange("p (g c) -> p g c", g=gn),
                        mybir.AxisListType.X, OP.add)
                ls_all = pc.tile([128, FW], F32, tag=f"ls{j}")
                nc.scalar.activation(ls_all[:], ss_all[:], AF.Ln)
                ost = pd.tile([128, FW, C], F32, tag=f"ost{j}", bufs=1)
                for fw in range(FW):
                    nc.vector.tensor_scalar(
                        ost[:, fw, :], xs_all[:, fw * C:(fw + 1) * C],
                        ls_all[:, fw:fw + 1], None, OP.subtract)
                nc.sync.dma_start(
                    out=outc[:, fcol:fcol + FW, :], in_=ost[:])
                fcol += FW

    nc.compile()
    return nc


_PROGRAM_CACHE = {}


def _get_program(cfg):
    key = (cfg.N, cfg.NWIN1, cfg.NB1, cfg.K1, cfg.W2, cfg.NRS)
    if key not in _PROGRAM_CACHE:
        _PROGRAM_CACHE[key] = build_program(cfg)
    return _PROGRAM_CACHE[key]


def kernel(x, W1, b1, W2, b2, edge_index):
    cfg, in_maps, owner, p, wcol = prep(x, W1, b1, W2, b2, edge_index, CFG_FULL)
    nc = _get_program(cfg)
    res = run_bass_kernel_spmd(
        nc, in_maps, core_ids=list(range(cfg.NC)),
        trace=bool(os.environ.get("GCN_TRACE")))
    if res.exec_time_ns is not None:
        print(f"HW exec time: {res.exec_time_ns} ns")
    outs = np.stack([res.results[k]["outc"] for k in range(cfg.NC)])
    return outs[owner, p, wcol].astype(np.float32)


# revision 75
# speedup vs baseline: 2.6269x; 1.0061x over previous
"""GCN (2-layer, symmetric-normalized A+I) on 8 Trainium2 NeuronCores.

Strategy:
  L1 (dst-sharded, no comm): x is replicated, so core k aggregates x[src] for
  its own 12500 dst nodes via one-hot matmul segment-sum (bin-packed windows
  of 128 nodes x 4 src-chunk cells), then dense W1/relu/W2, scaling rows by
  dinv[d] into h2s_c (local compact order, rows padded to 256B for gather).

  L2 (src-sharded partials + ReduceScatter): core k owns h2 for its own nodes
  only.  It processes exactly the edges whose SRC it owns (plus self-loops of
  its own nodes), gathering from LOCAL h2s_c, and accumulates weighted
  one-hot matmul partial sums over a GLOBAL window layout shared by all cores
  (host packs all N nodes into W2 windows of <=128 nodes with per-core edge
  load <= 128).  The [W2*128, 64] bf16 partial buffers are then summed across
  cores with two chunked ReduceScatter(+add) collectives (output = 1/8th of
  the data => cheap), and each core finishes log_softmax on the compact rows
  it receives.  The host maps compact rows back to natural node order.

  Gathers are declared uint32 (256B rows = 64 x u32) and bitcast back to bf16
  for the PE, which halves the gather element count.

kernel(**inputs) takes FULL inputs and returns the FULL [100000, 64] output.
"""
import os
import numpy as np

os.environ.setdefault("NEURON_SCRATCHPAD_PAGE_SIZE", "256")

import concourse.bacc as bacc
import concourse.tile as tile
import concourse.mybir as mybir
from concourse import bass
from concourse.bass_utils import run_bass_kernel_spmd

F32 = mybir.dt.float32
BF16 = mybir.dt.bfloat16
I16 = mybir.dt.int16
U32 = mybir.dt.uint32
U64 = mybir.dt.uint64
NPBF16 = mybir.dt.np(BF16)

AF = mybir.ActivationFunctionType
OP = mybir.AluOpType


class Cfg:
    def __init__(self, N=100000, R=12500, F=128, HID=256, C=64, NC=8,
                 NWIN1=100, NB1=4, K1=256, W2=784, NRS=2):
        self.N, self.R, self.F, self.HID, self.C, self.NC = N, R, F, HID, C, NC
        self.NWIN1, self.NB1, self.K1 = NWIN1, NB1, K1
        self.NBATCH1 = NWIN1 // NB1
        assert NB1 * K1 <= 1024 and (NB1 * K1) % 128 == 0
        self.NCOL1 = NB1 * K1 // 128
        self.IDX_REAL = NB1 * K1
        self.IDX_SELF = NB1 * 128
        self.CH1 = N // 4
        self.CROWS = NWIN1 * 128          # h2s rows per core
        assert self.CH1 <= 32768 and self.CROWS <= 32768
        # L1 column->cell map (no splits for K1=256)
        self.colmap = []
        for j in range(self.NCOL1):
            a = (j * 128) // K1
            self.colmap.append((a, (j * 128 + 127) // K1 != a))
        self.GIDX1_B = 4 * self.IDX_REAL // 16 + self.IDX_SELF // 16
        self.DRA1_B = 4 * self.NCOL1 + NB1
        self.DRB1_B = 4 * self.NCOL1
        # L2
        self.W2 = W2
        self.NB2 = 8
        self.K2 = 128
        assert W2 % (self.NB2 * NRS) == 0 and W2 % 16 == 0
        self.CALLS2 = W2 // self.NB2
        self.NRS = NRS
        # window split across RS chunks (large first, small last to shrink
        # the post-L2 tail); each chunk gets its own partial tensor
        big = (W2 * 7 // 10 + 7) // 8 * 8
        self.WSPLIT = [big, W2 - big] if NRS == 2 else [W2]
        assert all(w % self.NB2 == 0 for w in self.WSPLIT)
        self.FWINS = [w * 128 // NC // 128 for w in self.WSPLIT]
        assert all(w * 128 % (NC * 128) == 0 for w in self.WSPLIT)
        self.FWIN_TOT = sum(self.FWINS)


CFG_FULL = Cfg()


# ----------------------------------------------------------------------------
# host prep
# ----------------------------------------------------------------------------

def _pack4(cnt, nwin, K, WN=128):
    """Bin-pack nodes (rows of cnt [n,4]) into nwin windows, per-cell cap K,
    count cap WN. Returns (assign, rank) or None."""
    degs = cnt.sum(1)
    order = np.argsort(-degs, kind="stable")
    loads = np.zeros((nwin, 4), np.int64)
    counts = np.zeros(nwin, np.int64)
    assign = np.full(len(degs), -1, np.int64)
    rank = np.full(len(degs), -1, np.int64)
    for i in order:
        c = cnt[i]
        ok = (counts < WN) & ((loads + c) <= K).all(axis=1)
        if not ok.any():
            return None
        score = (loads + c).max(axis=1) * 1000 + counts
        score = np.where(ok, score, 1 << 40)
        w = int(np.argmin(score))
        assign[i] = w
        rank[i] = counts[w]
        counts[w] += 1
        loads[w] += c
    return assign, rank


def _pack8(cnt8, W, K=128, WN=128):
    """Global 8-dim pack: all N nodes into W windows; per-core load cap K,
    count cap WN. Returns (assign, rank) or None."""
    n = len(cnt8)
    tot = cnt8.sum(1)
    order = np.argsort(-tot, kind="stable")
    loads = np.zeros((W, 8), np.int32)
    counts = np.zeros(W, np.int32)
    assign = np.full(n, -1, np.int64)
    rank = np.full(n, -1, np.int64)
    big = np.int64(1) << 40
    for i in order:
        c = cnt8[i]
        nl = loads + c[None, :]
        ok = (counts < WN) & (nl <= K).all(axis=1)
        if not ok.any():
            return None
        score = nl.max(axis=1).astype(np.int64) * 256 + counts
        score = np.where(ok, score, big)
        w = int(np.argmin(score))
        assign[i] = w
        rank[i] = counts[w]
        counts[w] += 1
        loads[w] = nl[w]
    return assign, rank


def _wrap_idx(a):
    """[..., n] -> [..., 128, n//16]; idx i at [i%16, i//16], replicated x8."""
    lead = a.shape[:-1]
    n = a.shape[-1]
    w = a.reshape(*lead, n // 16, 16)
    w = np.moveaxis(w, -1, -2)
    return np.tile(w, (*([1] * len(lead)), 8, 1)).astype(np.int16)


def _wrap_col(a, dt):
    """[..., n] -> [..., 128, n//128]; slot p at [p%128, p//128]."""
    lead = a.shape[:-1]
    n = a.shape[-1]
    w = a.reshape(*lead, n // 128, 128)
    return np.ascontiguousarray(np.moveaxis(w, -1, -2)).astype(dt)


def _emit_l1(cfg, dl, gsrc, we, self_w, assign, rank):
    """Per-core L1 streams. dl/gsrc/we: local dst, global src, edge weight.
    self gather index = local node id (into x_own); self_w per local node."""
    NB, K, NCOL, NBATCH = cfg.NB1, cfg.K1, cfg.NCOL1, cfg.NBATCH1
    IDX_REAL, IDX_SELF = cfg.IDX_REAL, cfg.IDX_SELF
    CH = cfg.CH1
    chunk = gsrc // CH
    w = assign[dl]
    r = rank[dl]
    key = w * 4 + chunk
    order = np.argsort(key, kind="stable")
    skey = key[order]
    cellcnt = np.bincount(skey, minlength=cfg.NWIN1 * 4)
    cellstart = np.concatenate([[0], np.cumsum(cellcnt)[:-1]])
    within = np.arange(len(order)) - cellstart[skey]
    assert within.max(initial=0) < K, f"L1 cell overflow {within.max()}"
    bb = w[order] // NB
    wl = w[order] % NB
    slot = wl * K + within
    ch = chunk[order]

    gidx = np.zeros((NBATCH, 4, IDX_REAL), np.int64)
    gwv = np.zeros((NBATCH, 4, IDX_REAL), np.float32)
    rk = np.full((NBATCH, 4, IDX_REAL), -1000.0, np.float32)
    gidx[bb, ch, slot] = gsrc[order] - ch * CH
    gwv[bb, ch, slot] = we[order]
    rk[bb, ch, slot] = r[order]

    slots = np.arange(IDX_REAL)
    cell_of = slots // K
    wlA = ((slots // 128) * 128) // K
    dra = np.where(cell_of[None, None, :] == wlA[None, None, :], rk, -1000.0)
    drb = np.where(cell_of[None, None, :] == (wlA + 1)[None, None, :], rk, -1000.0)

    # self call
    node_at = np.full((cfg.NWIN1, 128), -1, np.int64)
    node_at[assign, rank] = np.arange(len(assign))
    sidx = np.zeros((NBATCH, IDX_SELF), np.int64)
    sw = np.zeros((NBATCH, IDX_SELF), np.float32)
    sdr = np.full((NBATCH, IDX_SELF), -1000.0, np.float32)
    rr = np.arange(IDX_SELF) % 128
    for b in range(NBATCH):
        flat = node_at[b * NB:(b + 1) * NB].reshape(-1)
        valid = flat >= 0
        sidx[b][valid] = flat[valid]
        sw[b][valid] = self_w[flat[valid]]
        sdr[b][valid] = rr[valid]

    gidx_parts, dra_parts, drb_parts, gw_parts = [], [], [], []
    for b in range(NBATCH):
        for c in range(4):
            gidx_parts.append(_wrap_idx(gidx[b, c]))
            dra_parts.append(_wrap_col(dra[b, c], np.float32))
            drb_parts.append(_wrap_col(drb[b, c], np.float32))
            gw_parts.append(_wrap_col(gwv[b, c], np.float32))
        gidx_parts.append(_wrap_idx(sidx[b]))
        dra_parts.append(_wrap_col(sdr[b], np.float32))
        gw_parts.append(_wrap_col(sw[b], np.float32))
    return {
        "gidx1": np.concatenate(gidx_parts, axis=1),
        "dra1": np.concatenate(dra_parts, axis=1),
        "drb1": np.concatenate(drb_parts, axis=1),
        "gw1": np.concatenate(gw_parts, axis=1),
    }


def _emit_l2(cfg, dst_g, gsrc_local, we, assign2, rank2):
    """Per-core L2 streams over the GLOBAL window layout."""
    W2, K = cfg.W2, cfg.K2
    w = assign2[dst_g]
    r = rank2[dst_g]
    order = np.argsort(w, kind="stable")
    sw = w[order]
    cnt = np.bincount(sw, minlength=W2)
    start = np.concatenate([[0], np.cumsum(cnt)[:-1]])
    within = np.arange(len(order)) - start[sw]
    assert within.max(initial=0) < K, f"L2 cell overflow {within.max()}"
    slot = sw * K + within

    nslots = W2 * K
    gidx = np.zeros(nslots, np.int64)
    gwv = np.zeros(nslots, np.float32)
    rk = np.full(nslots, -1000.0, np.float32)
    gidx[slot] = gsrc_local[order]
    gwv[slot] = we[order]
    rk[slot] = r[order]

    ncalls = cfg.CALLS2
    per = cfg.NB2 * K
    return {
        "gidx2": np.concatenate(
            [_wrap_idx(gidx.reshape(ncalls, per)[i]) for i in range(ncalls)], axis=1),
        "dra2": np.concatenate(
            [_wrap_col(rk.reshape(ncalls, per)[i], np.float32) for i in range(ncalls)], axis=1),
        "gw2": np.concatenate(
            [_wrap_col(gwv.reshape(ncalls, per)[i], np.float32) for i in range(ncalls)], axis=1),
    }


def prep(x, W1, b1, W2, b2, edge_index, cfg):
    N, R, NC = cfg.N, cfg.R, cfg.NC
    src = np.asarray(edge_index[0], np.int64)
    dst = np.asarray(edge_index[1], np.int64)
    x_bf = np.ascontiguousarray(np.asarray(x, np.float32).astype(NPBF16))
    deg = np.ones(N, np.float32)
    np.add.at(deg, dst, 1.0)
    dinv = (1.0 / np.sqrt(deg)).astype(np.float32)

    # ---- L1: per-dst-owner edges
    owner_d = dst // R
    per_core_l1 = []
    for k in range(NC):
        m = owner_d == k
        per_core_l1.append((src[m], dst[m] - k * R))

    pk1 = []
    for k in range(NC):
        s_k, d_k = per_core_l1[k]
        cnt = np.zeros((R, 4), np.int64)
        np.add.at(cnt, (d_k, s_k // cfg.CH1), 1)
        r = _pack4(cnt, cfg.NWIN1, cfg.K1)
        assert r is not None, f"L1 packing failed core {k}"
        pk1.append(r)

    # local compact position of every node (within its owner's h2s_c,
    # r-major layout: row = rank * NWIN1 + window)
    cpos_local = np.empty(N, np.int64)
    for k in range(NC):
        a1, r1 = pk1[k]
        cpos_local[k * R:(k + 1) * R] = r1 * cfg.NWIN1 + a1

    # ---- L2: global window packing on per-src-owner dst counts (+ self)
    owner_s = src // R
    cnt8 = np.zeros((N, NC), np.int32)
    np.add.at(cnt8, (dst, owner_s), 1)
    cnt8[np.arange(N), np.arange(N) // R] += 1
    pk2 = None
    for W2G in (cfg.W2, cfg.W2 + 16, cfg.W2 + 32, cfg.W2 + 64):
        if W2G != cfg.W2:
            cfg = Cfg(W2=W2G)
        pk2 = _pack8(cnt8, cfg.W2, cfg.K2)
        if pk2 is not None:
            break
    assert pk2 is not None, "L2 global packing failed"
    a2, r2 = pk2

    iota = np.tile(np.arange(128, dtype=np.float32), (128, 1)).astype(NPBF16)
    b1w = np.zeros((128, cfg.HID // 128), np.float32)
    for h in range(cfg.HID):
        b1w[h % 128, h // 128] = b1[h]
    b2bc = np.tile(np.asarray(b2, np.float32), (128, 8)).astype(NPBF16)

    in_maps = []
    for k in range(NC):
        s_k, d_k = per_core_l1[k]
        a1, r1 = pk1[k]
        dloc = dinv[k * R:(k + 1) * R]

        st1 = _emit_l1(cfg, d_k, s_k, dinv[s_k] * dloc[d_k], dloc * dloc, a1, r1)

        # L2 edges: src owned by k (+ self loops of k's nodes)
        m = owner_s == k
        l2_dst = np.concatenate([dst[m], np.arange(k * R, (k + 1) * R)])
        l2_srcl = np.concatenate([cpos_local[src[m]], cpos_local[k * R:(k + 1) * R]])
        l2_we = dinv[np.concatenate([dst[m], np.arange(k * R, (k + 1) * R)])]
        st2 = _emit_l2(cfg, l2_dst, l2_srcl, l2_we, a2, r2)

        dc = np.zeros((cfg.NWIN1, 128), np.float32)
        dc[a1, r1] = dloc

        in_maps.append({
            "x": x_bf,
            "x_own": np.ascontiguousarray(x_bf[k * R:(k + 1) * R]),
            "W1": np.asarray(W1, np.float32),
            "W2w": np.asarray(W2, np.float32),
            "b1w": b1w,
            "b2bc": b2bc,
            "iota": iota,
            "dinv1": np.ascontiguousarray(dc.T),
            **st1,
            **st2,
        })

    # host map: node -> (owner core, outc partition p, outc column wcol)
    # partial_j is r-major [128, Wj, C] over its window subrange; RS chunk j
    # scatters flat (rank, window) rows; core k gets ranks [16k, 16k+16).
    wsplit = np.asarray(cfg.WSPLIT)
    wstart = np.concatenate([[0], np.cumsum(wsplit)[:-1]])
    fstart = np.concatenate([[0], np.cumsum(cfg.FWINS)[:-1]])
    j = np.searchsorted(np.cumsum(wsplit), a2, side="right")
    wj = a2 - wstart[j]
    RPC = 128 // cfg.NC
    owner = r2 // RPC
    i = (r2 % RPC) * wsplit[j] + wj
    p = i % 128
    wcol = fstart[j] + i // 128
    return cfg, in_maps, owner, p, wcol


# ----------------------------------------------------------------------------
# device program
# ----------------------------------------------------------------------------

def build_program(cfg):
    nc = bacc.Bacc(None, target_bir_lowering=False, debug=False)
    F, HID, C = cfg.F, cfg.HID, cfg.C
    NB1, NCOL1, NBATCH1 = cfg.NB1, cfg.NCOL1, cfg.NBATCH1

    x = nc.declare_dram_parameter("x", [cfg.N, F], BF16, isOutput=False)
    x_own = nc.declare_dram_parameter("x_own", [cfg.R, F], BF16, isOutput=False)
    W1p = nc.declare_dram_parameter("W1", [F, HID], F32, isOutput=False)
    W2p = nc.declare_dram_parameter("W2w", [HID, C], F32, isOutput=False)
    b1w = nc.declare_dram_parameter("b1w", [128, HID // 128], F32, isOutput=False)
    b2bc = nc.declare_dram_parameter("b2bc", [128, 8 * C], BF16, isOutput=False)
    iota = nc.declare_dram_parameter("iota", [128, 128], BF16, isOutput=False)
    dinv1 = nc.declare_dram_parameter("dinv1", [128, cfg.NWIN1], F32, isOutput=False)
    gidx1 = nc.declare_dram_parameter("gidx1", [128, NBATCH1 * cfg.GIDX1_B], I16, isOutput=False)
    dra1 = nc.declare_dram_parameter("dra1", [128, NBATCH1 * cfg.DRA1_B], F32, isOutput=False)
    drb1 = nc.declare_dram_parameter("drb1", [128, NBATCH1 * cfg.DRB1_B], F32, isOutput=False)
    gw1 = nc.declare_dram_parameter("gw1", [128, NBATCH1 * cfg.DRA1_B], F32, isOutput=False)
    gidx2 = nc.declare_dram_parameter("gidx2", [128, cfg.CALLS2 * 64], I16, isOutput=False)
    dra2 = nc.declare_dram_parameter("dra2", [128, cfg.W2], F32, isOutput=False)
    gw2 = nc.declare_dram_parameter("gw2", [128, cfg.W2], F32, isOutput=False)
    outc = nc.declare_dram_parameter(
        "outc", [128, cfg.FWIN_TOT, C], F32, isOutput=True)

    # r-major 3D layouts so staging DMAs are contiguous per partition
    h2s_c = nc.dram_tensor("h2s_c", [128, cfg.NWIN1, F], BF16)
    partials = [nc.dram_tensor(f"partial{j}", [128, w, C], BF16)
                for j, w in enumerate(cfg.WSPLIT)]
    rs_out = [nc.dram_tensor(f"rs_out{j}", [fw * 128, C], BF16)
              for j, fw in enumerate(cfg.FWINS)]

    with tile.TileContext(nc) as tc:
        with (
            tc.tile_pool(name="const", bufs=1) as pc,
            tc.tile_pool(name="gpool", bufs=3) as pg,
            tc.tile_pool(name="spool", bufs=2) as ps,
            tc.tile_pool(name="dense", bufs=2) as pd,
            tc.tile_pool(name="psagg", bufs=2, space="PSUM") as ppa,
            tc.tile_pool(name="psdense", bufs=2, space="PSUM") as ppd,
            tc.tile_pool(name="psl2", bufs=2, space="PSUM") as ppl,
        ):
            def load(param, shape, dtype, tag):
                t = pc.tile(shape, dtype, tag=tag)
                nc.sync.dma_start(out=t[:], in_=param[:, :])
                return t

            # L1-critical streams first so the first gathers start ASAP
            streams = {}
            for nm, par, wid, dt in (
                ("gidx1", gidx1, NBATCH1 * cfg.GIDX1_B, I16),
                ("dra1", dra1, NBATCH1 * cfg.DRA1_B, F32),
                ("gw1", gw1, NBATCH1 * cfg.DRA1_B, F32),
            ):
                streams[nm] = load(par, [128, wid], dt, "st_" + nm)
            iota_sb = load(iota, [128, 128], BF16, "iotasb")
            W1f = load(W1p, [F, HID], F32, "W1f")
            W2fa = pc.tile([128, C], F32, tag="W2fa")
            W2fb = pc.tile([128, C], F32, tag="W2fb")
            nc.sync.dma_start(out=W2fa[:], in_=W2p[0:128, :])
            nc.sync.dma_start(out=W2fb[:], in_=W2p[128:256, :])
            b1_sb = load(b1w, [128, HID // 128], F32, "b1sb")
            b2_sb = load(b2bc, [128, 8 * C], BF16, "b2sb")
            dinv1_sb = load(dinv1, [128, cfg.NWIN1], F32, "dinv1sb")

            W1_d = pc.tile([F, HID], BF16, tag="W1d")
            nc.vector.tensor_copy(W1_d[:], W1f[:])
            W2a_d = pc.tile([128, C], BF16, tag="W2ad")
            W2b_d = pc.tile([128, C], BF16, tag="W2bd")
            nc.vector.tensor_copy(W2a_d[:], W2fa[:])
            nc.vector.tensor_copy(W2b_d[:], W2fb[:])

            for nm, par, wid, dt in (
                ("drb1", drb1, NBATCH1 * cfg.DRB1_B, F32),
                ("gidx2", gidx2, cfg.CALLS2 * 64, I16),
                ("dra2", dra2, cfg.W2, F32),
                ("gw2", gw2, cfg.W2, F32),
            ):
                streams[nm] = load(par, [128, wid], dt, "st_" + nm)

            # two persistent h2s staging tiles, pad columns zeroed once
            hstages = []
            for i in range(2):
                t = pc.tile([128, NB1, F], BF16, tag=f"hst{i}")
                nc.vector.memset(t[:], 0.0)
                hstages.append(t)

            # pin the one act table containing Relu/Copy/Exp/Ln so no
            # mid-kernel table reloads happen
            warm = pc.tile([128, 1], F32, tag="warm")
            nc.scalar.activation(warm[:], b1_sb[:, 0:1], AF.Ln)
            nc.scalar.activation(warm[:], b1_sb[:, 0:1], AF.Exp)

            # ---------------- L1 ----------------
            gi1, da1, db1, gwt1 = (streams["gidx1"], streams["dra1"],
                                   streams["drb1"], streams["gw1"])
            from concourse.bass import _add_dep_helper as _adh

            def _add_dep_l1(a, b):
                _adh(a.ins, b.ins, sync=True, reason="p2 bank-zero after relu_b")

            def dense_l1(b, bank):
                aggT = pd.tile([128, NB1 * 128], BF16, tag="aggT", bufs=3)
                nc.scalar.activation(aggT[:], bank[:], AF.Copy)
                hstage = hstages[b % 2]
                for wl in range(NB1):
                    w = b * NB1 + wl
                    a_sl = aggT[:, wl * 128:(wl + 1) * 128]
                    # ph and p2 share one PSUM bank tile: p2's start=True
                    # zeroes the whole bank, so p2's first matmul must wait
                    # for BOTH relus (the hT dep covers relu_a; relu_b is
                    # pinned explicitly below)
                    phb = ppd.tile([128, HID + C], F32, tag="ph", bufs=3)
                    ph = phb[:, 0:HID]
                    nc.tensor.matmul(ph[:, 0:128], lhsT=W1_d[:, 0:128],
                                     rhs=a_sl, start=True, stop=False,
                                     skip_group_check=True)
                    nc.tensor.matmul(ph[:, 128:256], lhsT=W1_d[:, 128:256],
                                     rhs=a_sl, start=False, stop=True,
                                     skip_group_check=True)
                    hT = pd.tile([128, HID], BF16, tag="hT", bufs=4)
                    nc.scalar.activation(hT[:, 0:128], ph[:, 0:128],
                                         AF.Relu, bias=b1_sb[:, 0:1])
                    relu_b = nc.scalar.activation(hT[:, 128:256], ph[:, 128:256],
                                                  AF.Relu, bias=b1_sb[:, 1:2])
                    p2 = phb[:, HID:HID + C]
                    mm1 = nc.tensor.matmul(p2, lhsT=hT[:, 0:128], rhs=W2a_d[:],
                                           start=True, stop=False,
                                           skip_group_check=True)
                    _add_dep_l1(mm1, relu_b)
                    nc.tensor.matmul(p2, lhsT=hT[:, 128:256], rhs=W2b_d[:],
                                     start=False, stop=True,
                                     skip_group_check=True)
                    nc.vector.tensor_scalar(hstage[:, wl, 0:C], p2,
                                            dinv1_sb[:, w:w + 1], None, OP.mult)
                nc.sync.dma_start(
                    out=h2s_c[:, b * NB1:(b + 1) * NB1, :], in_=hstage[:])

            for b in range(NBATCH1):
                go = b * cfg.GIDX1_B
                ao = b * cfg.DRA1_B
                bank = ppa.tile([128, NB1 * 128], F32, tag="aggps")

                first = True
                for c in range(4):
                    G = pg.tile([128, NCOL1, F // 2], U32, tag="G1", bufs=6)
                    nc.gpsimd.dma_gather(
                        G[:], x[c * cfg.CH1:(c + 1) * cfg.CH1, :].bitcast(U32),
                        gi1[:, go + c * (cfg.IDX_REAL // 16):
                            go + (c + 1) * (cfg.IDX_REAL // 16)],
                        cfg.IDX_REAL, cfg.IDX_REAL, F // 2)
                    Gbf = G[:].bitcast(BF16)
                    for j, (wa, split) in enumerate(cfg.colmap):
                        for wl, second in ((wa, False), (wa + 1, True)) if split else ((wa, False),):
                            if wl >= NB1:
                                continue
                            col = ao + c * NCOL1 + j
                            S = ps.tile([128, 128], BF16, tag="S", bufs=10)
                            dsrc = db1 if second else da1
                            dcol = (b * cfg.DRB1_B + c * NCOL1 + j) if second else col
                            eng = nc.gpsimd if c == 3 else nc.vector
                            eng.tensor_scalar(
                                S[:], iota_sb[:],
                                dsrc[:, dcol:dcol + 1], gwt1[:, col:col + 1],
                                OP.is_equal, OP.mult)
                            nc.tensor.matmul(
                                bank[:, wl * 128:(wl + 1) * 128],
                                lhsT=Gbf[:, j, :], rhs=S[:],
                                start=first, stop=False, skip_group_check=True)
                            first = False
                # self
                Gs = pg.tile([128, NB1, F // 2], U32, tag="Gs1", bufs=4)
                so = go + 4 * (cfg.IDX_REAL // 16)
                nc.gpsimd.dma_gather(
                    Gs[:], x_own[:, :].bitcast(U32),
                    gi1[:, so:so + cfg.IDX_SELF // 16],
                    cfg.IDX_SELF, cfg.IDX_SELF, F // 2)
                Gsbf = Gs[:].bitcast(BF16)
                for wl in range(NB1):
                    col = ao + 4 * NCOL1 + wl
                    S = ps.tile([128, 128], BF16, tag="S", bufs=10)
                    nc.gpsimd.tensor_scalar(
                        S[:], iota_sb[:],
                        da1[:, col:col + 1], gwt1[:, col:col + 1],
                        OP.is_equal, OP.mult)
                    nc.tensor.matmul(
                        bank[:, wl * 128:(wl + 1) * 128],
                        lhsT=Gsbf[:, wl, :], rhs=S[:],
                        start=False, stop=(wl == NB1 - 1),
                        skip_group_check=True)
                dense_l1(b, bank)

            # ---------------- L2: src-sharded partials ----------------
            gi2, da2, gwt2 = streams["gidx2"], streams["dra2"], streams["gw2"]
            NB2, K2 = cfg.NB2, cfg.K2
            call_split = [w // NB2 for w in cfg.WSPLIT]

            last_gather = None
            for call in range(cfg.CALLS2):
                G2 = pg.tile([128, NB2, F // 2], U32, tag="G2", bufs=26)
                last_gather = nc.gpsimd.dma_gather(
                    G2[:],
                    h2s_c[:, :, :].rearrange("r w f -> (r w) f").bitcast(U32),
                    gi2[:, call * 64:(call + 1) * 64],
                    NB2 * K2, NB2 * K2, F // 2)
                G2bf = G2[:].bitcast(BF16)
                bank2 = ppl.tile([128, NB2 * C], F32, tag="l2ps", bufs=3)
                for wl in range(NB2):
                    w = call * NB2 + wl
                    S = ps.tile([128, 128], BF16, tag="S2", bufs=16)
                    eng = nc.vector
                    si = eng.tensor_scalar(
                        S[:], iota_sb[:],
                        da2[:, w:w + 1], gwt2[:, w:w + 1],
                        OP.is_equal, OP.mult)
                    nc.tensor.matmul(
                        bank2[:, wl * C:(wl + 1) * C],
                        lhsT=S[:], rhs=G2bf[:, wl, 0:C],
                        start=(wl == 0), stop=(wl == NB2 - 1),
                        skip_group_check=True)
                stage = pd.tile([128, NB2, C], BF16, tag="pstage", bufs=6)
                nc.scalar.activation(stage[:], bank2[:], AF.Copy)
                pj = 0 if call < call_split[0] else 1
                pcall = call - (0 if pj == 0 else call_split[0])
                last_pdma = nc.sync.dma_start(
                    out=partials[pj][:, pcall * NB2:(pcall + 1) * NB2, :],
                    in_=stage[:])

            # Collectives and the rs_out loads all live on the Pool queue,
            # explicitly chained (RS0 -> lt0 -> RS1 -> lt1) and pinned after
            # the last gather: the tile scheduler otherwise hoists the
            # collectives and stalls the L2 pipeline behind them.
            from concourse.bass import _add_dep_helper
            lts = []
            prev = last_gather
            dprev = last_pdma
            for j in range(cfg.NRS):
                cc = nc.gpsimd.collective_compute(
                    "ReduceScatter", OP.add,
                    ins=[partials[j][:, :, :]],
                    outs=[rs_out[j][:, :]],
                    replica_groups=[list(range(cfg.NC))],
                )
                _add_dep_helper(cc.ins, prev.ins, sync=True,
                                reason="keep Pool queue order")
                prev = cc
                FW = cfg.FWINS[j]
                lt = pd.tile([128, FW, C], BF16, tag=f"ltall{j}", bufs=1)
                ld = nc.sync.dma_start(
                    out=lt[:],
                    in_=rs_out[j][:, :].rearrange("(w r) c -> r w c", w=FW))
                _add_dep_helper(ld.ins, dprev.ins, sync=True,
                                reason="keep SP queue order")
                lts.append(lt)
                dprev = ld

            # ---------------- final: +b2, log_softmax on received rows ----
            # (all on DVE/ACT/SP: Pool is busy with the collectives)
            fcol = 0
            for j in range(cfg.NRS):
                FW = cfg.FWINS[j]
                lt = lts[j]
                # logits are O(+-40) so exp cannot overflow f32: skip the
                # usual max-shift and compute log_softmax = x - ln(sum(exp x)),
                # batching 8 windows per op
                xs_all = pc.tile([128, FW * C], F32, tag=f"xs{j}")
                ss_all = pc.tile([128, FW], F32, tag=f"ss{j}")
                for g0 in range(0, FW, 8):
                    gn = min(8, FW - g0)
                    xs = xs_all[:, g0 * C:(g0 + gn) * C]
                    nc.vector.tensor_add(
                        xs, lt[:, g0:g0 + gn, :], b2_sb[:, 0:gn * C])
                    es = pd.tile([128, 8 * C], F32, tag="es")
                    nc.scalar.activation(es[:, 0:gn * C], xs, AF.Exp)
                    nc.vector.tensor_reduce(
                        ss_all[:, g0:g0 + gn],
                        es[:, 0:gn * C].rearrange("p (g c) -> p g c", g=gn),
                        mybir.AxisListType.X, OP.add)
                ls_all = pc.tile([128, FW], F32, tag=f"ls{j}")
                nc.scalar.activation(ls_all[:], ss_all[:], AF.Ln)
                ost = pd.tile([128, FW, C], F32, tag=f"ost{j}", bufs=1)
                for fw in range(FW):
                    nc.vector.tensor_scalar(
                        ost[:, fw, :], xs_all[:, fw * C:(fw + 1) * C],
                        ls_all[:, fw:fw + 1], None, OP.subtract)
                nc.sync.dma_start(
                    out=outc[:, fcol:fcol + FW, :], in_=ost[:])
                fcol += FW

    nc.compile()
    return nc


_PROGRAM_CACHE = {}


def _get_program(cfg):
    key = (cfg.N, cfg.NWIN1, cfg.NB1, cfg.K1, cfg.W2, cfg.NRS)
    if key not in _PROGRAM_CACHE:
        _PROGRAM_CACHE[key] = build_program(cfg)
    return _PROGRAM_CACHE[key]


def kernel(x, W1, b1, W2, b2, edge_index):
    cfg, in_maps, owner, p, wcol = prep(x, W1, b1, W2, b2, edge_index, CFG_FULL)
    nc = _get_program(cfg)
    res = run_bass_kernel_spmd(
        nc, in_maps, core_ids=list(range(cfg.NC)),
        trace=bool(os.environ.get("GCN_TRACE")))
    if res.exec_time_ns is not None:
        print(f"HW exec time: {res.exec_time_ns} ns")
    outs = np.stack([res.results[k]["outc"] for k in range(cfg.NC)])
    return outs[owner, p, wcol].astype(np.float32)


# revision 84
# speedup vs baseline: 2.6481x; 1.0080x over previous
"""GCN (2-layer, symmetric-normalized A+I) on 8 Trainium2 NeuronCores.

Strategy:
  L1 (dst-sharded, no comm): x is replicated, so core k aggregates x[src] for
  its own 12500 dst nodes via one-hot matmul segment-sum (bin-packed windows
  of 128 nodes x 4 src-chunk cells), then dense W1/relu/W2, scaling rows by
  dinv[d] into h2s_c (local compact order, rows padded to 256B for gather).

  L2 (src-sharded partials + ReduceScatter): core k owns h2 for its own nodes
  only.  It processes exactly the edges whose SRC it owns (plus self-loops of
  its own nodes), gathering from LOCAL h2s_c, and accumulates weighted
  one-hot matmul partial sums over a GLOBAL window layout shared by all cores
  (host packs all N nodes into W2 windows of <=128 nodes with per-core edge
  load <= 128).  The [W2*128, 64] bf16 partial buffers are then summed across
  cores with two chunked ReduceScatter(+add) collectives (output = 1/8th of
  the data => cheap), and each core finishes log_softmax on the compact rows
  it receives.  The host maps compact rows back to natural node order.

  Gathers are declared uint32 (256B rows = 64 x u32) and bitcast back to bf16
  for the PE, which halves the gather element count.

kernel(**inputs) takes FULL inputs and returns the FULL [100000, 64] output.
"""
import os
import numpy as np

os.environ.setdefault("NEURON_SCRATCHPAD_PAGE_SIZE", "256")

import concourse.bacc as bacc
import concourse.tile as tile
import concourse.mybir as mybir
from concourse import bass
from concourse.bass_utils import run_bass_kernel_spmd

F32 = mybir.dt.float32
BF16 = mybir.dt.bfloat16
I16 = mybir.dt.int16
U32 = mybir.dt.uint32
U64 = mybir.dt.uint64
NPBF16 = mybir.dt.np(BF16)

AF = mybir.ActivationFunctionType
OP = mybir.AluOpType


class Cfg:
    def __init__(self, N=100000, R=12500, F=128, HID=256, C=64, NC=8,
                 NWIN1=100, NB1=4, K1=256, W2=784, NRS=2):
        self.N, self.R, self.F, self.HID, self.C, self.NC = N, R, F, HID, C, NC
        self.NWIN1, self.NB1, self.K1 = NWIN1, NB1, K1
        self.NBATCH1 = NWIN1 // NB1
        assert NB1 * K1 <= 1024 and (NB1 * K1) % 128 == 0
        self.NCOL1 = NB1 * K1 // 128
        self.IDX_REAL = NB1 * K1
        self.IDX_SELF = NB1 * 128
        self.CH1 = N // 4
        self.CROWS = NWIN1 * 128          # h2s rows per core
        assert self.CH1 <= 32768 and self.CROWS <= 32768
        # L1 column->cell map (no splits for K1=256)
        self.colmap = []
        for j in range(self.NCOL1):
            a = (j * 128) // K1
            self.colmap.append((a, (j * 128 + 127) // K1 != a))
        self.GIDX1_B = 4 * self.IDX_REAL // 16 + self.IDX_SELF // 16
        self.DRA1_B = 4 * self.NCOL1 + NB1
        self.DRB1_B = 4 * self.NCOL1
        # L2
        self.W2 = W2
        self.NB2 = 8
        self.K2 = 128
        assert W2 % (self.NB2 * NRS) == 0 and W2 % 16 == 0
        self.CALLS2 = W2 // self.NB2
        self.NRS = NRS
        # window split across RS chunks (large first, small last to shrink
        # the post-L2 tail); each chunk gets its own partial tensor
        big = (W2 * 7 // 10 + 7) // 8 * 8
        self.WSPLIT = [big, W2 - big] if NRS == 2 else [W2]
        assert all(w % self.NB2 == 0 for w in self.WSPLIT)
        self.FWINS = [w * 128 // NC // 128 for w in self.WSPLIT]
        assert all(w * 128 % (NC * 128) == 0 for w in self.WSPLIT)
        self.FWIN_TOT = sum(self.FWINS)


CFG_FULL = Cfg()


# ----------------------------------------------------------------------------
# host prep
# ----------------------------------------------------------------------------

def _pack4(cnt, nwin, K, WN=128):
    """Bin-pack nodes (rows of cnt [n,4]) into nwin windows, per-cell cap K,
    count cap WN. Returns (assign, rank) or None."""
    degs = cnt.sum(1)
    order = np.argsort(-degs, kind="stable")
    loads = np.zeros((nwin, 4), np.int64)
    counts = np.zeros(nwin, np.int64)
    assign = np.full(len(degs), -1, np.int64)
    rank = np.full(len(degs), -1, np.int64)
    for i in order:
        c = cnt[i]
        ok = (counts < WN) & ((loads + c) <= K).all(axis=1)
        if not ok.any():
            return None
        score = (loads + c).max(axis=1) * 1000 + counts
        score = np.where(ok, score, 1 << 40)
        w = int(np.argmin(score))
        assign[i] = w
        rank[i] = counts[w]
        counts[w] += 1
        loads[w] += c
    return assign, rank


def _pack8(cnt8, W, K=128, WN=128):
    """Global 8-dim pack: all N nodes into W windows; per-core load cap K,
    count cap WN. Returns (assign, rank) or None."""
    n = len(cnt8)
    tot = cnt8.sum(1)
    order = np.argsort(-tot, kind="stable")
    loads = np.zeros((W, 8), np.int32)
    counts = np.zeros(W, np.int32)
    assign = np.full(n, -1, np.int64)
    rank = np.full(n, -1, np.int64)
    big = np.int64(1) << 40
    for i in order:
        c = cnt8[i]
        nl = loads + c[None, :]
        ok = (counts < WN) & (nl <= K).all(axis=1)
        if not ok.any():
            return None
        score = nl.max(axis=1).astype(np.int64) * 256 + counts
        score = np.where(ok, score, big)
        w = int(np.argmin(score))
        assign[i] = w
        rank[i] = counts[w]
        counts[w] += 1
        loads[w] = nl[w]
    return assign, rank


def _wrap_idx(a):
    """[..., n] -> [..., 128, n//16]; idx i at [i%16, i//16], replicated x8."""
    lead = a.shape[:-1]
    n = a.shape[-1]
    w = a.reshape(*lead, n // 16, 16)
    w = np.moveaxis(w, -1, -2)
    return np.tile(w, (*([1] * len(lead)), 8, 1)).astype(np.int16)


def _wrap_col(a, dt):
    """[..., n] -> [..., 128, n//128]; slot p at [p%128, p//128]."""
    lead = a.shape[:-1]
    n = a.shape[-1]
    w = a.reshape(*lead, n // 128, 128)
    return np.ascontiguousarray(np.moveaxis(w, -1, -2)).astype(dt)


def _emit_l1(cfg, dl, gsrc, we, self_w, assign, rank):
    """Per-core L1 streams. dl/gsrc/we: local dst, global src, edge weight.
    self gather index = local node id (into x_own); self_w per local node."""
    NB, K, NCOL, NBATCH = cfg.NB1, cfg.K1, cfg.NCOL1, cfg.NBATCH1
    IDX_REAL, IDX_SELF = cfg.IDX_REAL, cfg.IDX_SELF
    CH = cfg.CH1
    chunk = gsrc // CH
    w = assign[dl]
    r = rank[dl]
    key = w * 4 + chunk
    order = np.argsort(key, kind="stable")
    skey = key[order]
    cellcnt = np.bincount(skey, minlength=cfg.NWIN1 * 4)
    cellstart = np.concatenate([[0], np.cumsum(cellcnt)[:-1]])
    within = np.arange(len(order)) - cellstart[skey]
    assert within.max(initial=0) < K, f"L1 cell overflow {within.max()}"
    bb = w[order] // NB
    wl = w[order] % NB
    slot = wl * K + within
    ch = chunk[order]

    gidx = np.zeros((NBATCH, 4, IDX_REAL), np.int64)
    gwv = np.zeros((NBATCH, 4, IDX_REAL), np.float32)
    rk = np.full((NBATCH, 4, IDX_REAL), -1000.0, np.float32)
    gidx[bb, ch, slot] = gsrc[order] - ch * CH
    gwv[bb, ch, slot] = we[order]
    rk[bb, ch, slot] = r[order]

    slots = np.arange(IDX_REAL)
    cell_of = slots // K
    wlA = ((slots // 128) * 128) // K
    dra = np.where(cell_of[None, None, :] == wlA[None, None, :], rk, -1000.0)
    drb = np.where(cell_of[None, None, :] == (wlA + 1)[None, None, :], rk, -1000.0)

    # self call
    node_at = np.full((cfg.NWIN1, 128), -1, np.int64)
    node_at[assign, rank] = np.arange(len(assign))
    sidx = np.zeros((NBATCH, IDX_SELF), np.int64)
    sw = np.zeros((NBATCH, IDX_SELF), np.float32)
    sdr = np.full((NBATCH, IDX_SELF), -1000.0, np.float32)
    rr = np.arange(IDX_SELF) % 128
    for b in range(NBATCH):
        flat = node_at[b * NB:(b + 1) * NB].reshape(-1)
        valid = flat >= 0
        sidx[b][valid] = flat[valid]
        sw[b][valid] = self_w[flat[valid]]
        sdr[b][valid] = rr[valid]

    gidx_parts, dra_parts, drb_parts, gw_parts = [], [], [], []
    for b in range(NBATCH):
        for c in range(4):
            gidx_parts.append(_wrap_idx(gidx[b, c]))
            dra_parts.append(_wrap_col(dra[b, c], np.float32))
            drb_parts.append(_wrap_col(drb[b, c], np.float32))
            gw_parts.append(_wrap_col(gwv[b, c], np.float32))
        gidx_parts.append(_wrap_idx(sidx[b]))
        dra_parts.append(_wrap_col(sdr[b], np.float32))
        gw_parts.append(_wrap_col(sw[b], np.float32))
    return {
        "gidx1": np.concatenate(gidx_parts, axis=1),
        "dra1": np.concatenate(dra_parts, axis=1),
        "drb1": np.concatenate(drb_parts, axis=1),
        "gw1": np.concatenate(gw_parts, axis=1),
    }


def _emit_l2(cfg, dst_g, gsrc_local, we, assign2, rank2):
    """Per-core L2 streams over the GLOBAL window layout."""
    W2, K = cfg.W2, cfg.K2
    w = assign2[dst_g]
    r = rank2[dst_g]
    order = np.argsort(w, kind="stable")
    sw = w[order]
    cnt = np.bincount(sw, minlength=W2)
    start = np.concatenate([[0], np.cumsum(cnt)[:-1]])
    within = np.arange(len(order)) - start[sw]
    assert within.max(initial=0) < K, f"L2 cell overflow {within.max()}"
    slot = sw * K + within

    nslots = W2 * K
    gidx = np.zeros(nslots, np.int64)
    gwv = np.zeros(nslots, np.float32)
    rk = np.full(nslots, -1000.0, np.float32)
    gidx[slot] = gsrc_local[order]
    gwv[slot] = we[order]
    rk[slot] = r[order]

    ncalls = cfg.CALLS2
    per = cfg.NB2 * K
    return {
        "gidx2": np.concatenate(
            [_wrap_idx(gidx.reshape(ncalls, per)[i]) for i in range(ncalls)], axis=1),
        "dra2": np.concatenate(
            [_wrap_col(rk.reshape(ncalls, per)[i], np.float32) for i in range(ncalls)], axis=1),
        "gw2": np.concatenate(
            [_wrap_col(gwv.reshape(ncalls, per)[i], np.float32) for i in range(ncalls)], axis=1),
    }


def prep(x, W1, b1, W2, b2, edge_index, cfg):
    N, R, NC = cfg.N, cfg.R, cfg.NC
    src = np.asarray(edge_index[0], np.int64)
    dst = np.asarray(edge_index[1], np.int64)
    x_bf = np.ascontiguousarray(np.asarray(x, np.float32).astype(NPBF16))
    deg = np.ones(N, np.float32)
    np.add.at(deg, dst, 1.0)
    dinv = (1.0 / np.sqrt(deg)).astype(np.float32)

    # ---- L1: per-dst-owner edges
    owner_d = dst // R
    per_core_l1 = []
    for k in range(NC):
        m = owner_d == k
        per_core_l1.append((src[m], dst[m] - k * R))

    pk1 = []
    for k in range(NC):
        s_k, d_k = per_core_l1[k]
        cnt = np.zeros((R, 4), np.int64)
        np.add.at(cnt, (d_k, s_k // cfg.CH1), 1)
        r = _pack4(cnt, cfg.NWIN1, cfg.K1)
        assert r is not None, f"L1 packing failed core {k}"
        pk1.append(r)

    # local compact position of every node (within its owner's h2s_c,
    # r-major layout: row = rank * NWIN1 + window)
    cpos_local = np.empty(N, np.int64)
    for k in range(NC):
        a1, r1 = pk1[k]
        cpos_local[k * R:(k + 1) * R] = r1 * cfg.NWIN1 + a1

    # ---- L2: global window packing on per-src-owner dst counts (+ self)
    owner_s = src // R
    cnt8 = np.zeros((N, NC), np.int32)
    np.add.at(cnt8, (dst, owner_s), 1)
    cnt8[np.arange(N), np.arange(N) // R] += 1
    pk2 = None
    for W2G in (cfg.W2, cfg.W2 + 16, cfg.W2 + 32, cfg.W2 + 64):
        if W2G != cfg.W2:
            cfg = Cfg(W2=W2G)
        pk2 = _pack8(cnt8, cfg.W2, cfg.K2)
        if pk2 is not None:
            break
    assert pk2 is not None, "L2 global packing failed"
    a2, r2 = pk2

    iota = np.tile(np.arange(128, dtype=np.float32), (128, 1)).astype(NPBF16)
    b1w = np.zeros((128, cfg.HID // 128), np.float32)
    for h in range(cfg.HID):
        b1w[h % 128, h // 128] = b1[h]
    b2bc = np.tile(np.asarray(b2, np.float32), (128, 8)).astype(NPBF16)

    in_maps = []
    for k in range(NC):
        s_k, d_k = per_core_l1[k]
        a1, r1 = pk1[k]
        dloc = dinv[k * R:(k + 1) * R]

        st1 = _emit_l1(cfg, d_k, s_k, dinv[s_k] * dloc[d_k], dloc * dloc, a1, r1)

        # L2 edges: src owned by k (+ self loops of k's nodes)
        m = owner_s == k
        l2_dst = np.concatenate([dst[m], np.arange(k * R, (k + 1) * R)])
        l2_srcl = np.concatenate([cpos_local[src[m]], cpos_local[k * R:(k + 1) * R]])
        l2_we = dinv[np.concatenate([dst[m], np.arange(k * R, (k + 1) * R)])]
        st2 = _emit_l2(cfg, l2_dst, l2_srcl, l2_we, a2, r2)

        dc = np.zeros((cfg.NWIN1, 128), np.float32)
        dc[a1, r1] = dloc

        in_maps.append({
            "x": x_bf,
            "x_own": np.ascontiguousarray(x_bf[k * R:(k + 1) * R]),
            "W1": np.asarray(W1, np.float32),
            "W2w": np.asarray(W2, np.float32),
            "b1w": b1w,
            "b2bc": b2bc,
            "iota": iota,
            "dinv1": np.ascontiguousarray(dc.T),
            **st1,
            **st2,
        })

    # host map: node -> (owner core, outc partition p, outc column wcol)
    # partial_j is r-major [128, Wj, C] over its window subrange; RS chunk j
    # scatters flat (rank, window) rows; core k gets ranks [16k, 16k+16).
    wsplit = np.asarray(cfg.WSPLIT)
    wstart = np.concatenate([[0], np.cumsum(wsplit)[:-1]])
    fstart = np.concatenate([[0], np.cumsum(cfg.FWINS)[:-1]])
    j = np.searchsorted(np.cumsum(wsplit), a2, side="right")
    wj = a2 - wstart[j]
    RPC = 128 // cfg.NC
    owner = r2 // RPC
    i = (r2 % RPC) * wsplit[j] + wj
    p = i % 128
    wcol = fstart[j] + i // 128
    return cfg, in_maps, owner, p, wcol


# ----------------------------------------------------------------------------
# device program
# ----------------------------------------------------------------------------

def build_program(cfg):
    nc = bacc.Bacc(None, target_bir_lowering=False, debug=False)
    F, HID, C = cfg.F, cfg.HID, cfg.C
    NB1, NCOL1, NBATCH1 = cfg.NB1, cfg.NCOL1, cfg.NBATCH1

    x = nc.declare_dram_parameter("x", [cfg.N, F], BF16, isOutput=False)
    x_own = nc.declare_dram_parameter("x_own", [cfg.R, F], BF16, isOutput=False)
    W1p = nc.declare_dram_parameter("W1", [F, HID], F32, isOutput=False)
    W2p = nc.declare_dram_parameter("W2w", [HID, C], F32, isOutput=False)
    b1w = nc.declare_dram_parameter("b1w", [128, HID // 128], F32, isOutput=False)
    b2bc = nc.declare_dram_parameter("b2bc", [128, 8 * C], BF16, isOutput=False)
    iota = nc.declare_dram_parameter("iota", [128, 128], BF16, isOutput=False)
    dinv1 = nc.declare_dram_parameter("dinv1", [128, cfg.NWIN1], F32, isOutput=False)
    gidx1 = nc.declare_dram_parameter("gidx1", [128, NBATCH1 * cfg.GIDX1_B], I16, isOutput=False)
    dra1 = nc.declare_dram_parameter("dra1", [128, NBATCH1 * cfg.DRA1_B], F32, isOutput=False)
    drb1 = nc.declare_dram_parameter("drb1", [128, NBATCH1 * cfg.DRB1_B], F32, isOutput=False)
    gw1 = nc.declare_dram_parameter("gw1", [128, NBATCH1 * cfg.DRA1_B], F32, isOutput=False)
    gidx2 = nc.declare_dram_parameter("gidx2", [128, cfg.CALLS2 * 64], I16, isOutput=False)
    dra2 = nc.declare_dram_parameter("dra2", [128, cfg.W2], F32, isOutput=False)
    gw2 = nc.declare_dram_parameter("gw2", [128, cfg.W2], F32, isOutput=False)
    outc = nc.declare_dram_parameter(
        "outc", [128, cfg.FWIN_TOT, C], F32, isOutput=True)

    # r-major 3D layouts so staging DMAs are contiguous per partition
    h2s_c = nc.dram_tensor("h2s_c", [128, cfg.NWIN1, F], BF16)
    partials = [nc.dram_tensor(f"partial{j}", [128, w, C], BF16)
                for j, w in enumerate(cfg.WSPLIT)]
    rs_out = [nc.dram_tensor(f"rs_out{j}", [fw * 128, C], BF16)
              for j, fw in enumerate(cfg.FWINS)]

    with tile.TileContext(nc) as tc:
        with (
            tc.tile_pool(name="const", bufs=1) as pc,
            tc.tile_pool(name="gpool", bufs=3) as pg,
            tc.tile_pool(name="spool", bufs=2) as ps,
            tc.tile_pool(name="dense", bufs=2) as pd,
            tc.tile_pool(name="psagg", bufs=2, space="PSUM") as ppa,
            tc.tile_pool(name="psdense", bufs=2, space="PSUM") as ppd,
            tc.tile_pool(name="psl2", bufs=2, space="PSUM") as ppl,
        ):
            def load(param, shape, dtype, tag):
                t = pc.tile(shape, dtype, tag=tag)
                nc.sync.dma_start(out=t[:], in_=param[:, :])
                return t

            # L1-critical streams first so the first gathers start ASAP
            streams = {}
            for nm, par, wid, dt in (
                ("gidx1", gidx1, NBATCH1 * cfg.GIDX1_B, I16),
                ("dra1", dra1, NBATCH1 * cfg.DRA1_B, F32),
                ("gw1", gw1, NBATCH1 * cfg.DRA1_B, F32),
            ):
                streams[nm] = load(par, [128, wid], dt, "st_" + nm)
            iota_sb = load(iota, [128, 128], BF16, "iotasb")
            W1f = load(W1p, [F, HID], F32, "W1f")
            W2fa = pc.tile([128, C], F32, tag="W2fa")
            W2fb = pc.tile([128, C], F32, tag="W2fb")
            nc.sync.dma_start(out=W2fa[:], in_=W2p[0:128, :])
            nc.sync.dma_start(out=W2fb[:], in_=W2p[128:256, :])
            b1_sb = load(b1w, [128, HID // 128], F32, "b1sb")
            b2_sb = load(b2bc, [128, 8 * C], BF16, "b2sb")
            dinv1_sb = load(dinv1, [128, cfg.NWIN1], F32, "dinv1sb")

            W1_d = pc.tile([F, HID], BF16, tag="W1d")
            nc.vector.tensor_copy(W1_d[:], W1f[:])
            W2a_d = pc.tile([128, C], BF16, tag="W2ad")
            W2b_d = pc.tile([128, C], BF16, tag="W2bd")
            nc.vector.tensor_copy(W2a_d[:], W2fa[:])
            nc.vector.tensor_copy(W2b_d[:], W2fb[:])

            for nm, par, wid, dt in (
                ("drb1", drb1, NBATCH1 * cfg.DRB1_B, F32),
                ("gidx2", gidx2, cfg.CALLS2 * 64, I16),
                ("dra2", dra2, cfg.W2, F32),
                ("gw2", gw2, cfg.W2, F32),
            ):
                streams[nm] = load(par, [128, wid], dt, "st_" + nm)

            # two persistent h2s staging tiles, pad columns zeroed once
            hstages = []
            for i in range(2):
                t = pc.tile([128, NB1, F], BF16, tag=f"hst{i}")
                nc.vector.memset(t[:], 0.0)
                hstages.append(t)

            # pin the one act table containing Relu/Copy/Exp/Ln so no
            # mid-kernel table reloads happen
            warm = pc.tile([128, 1], F32, tag="warm")
            nc.scalar.activation(warm[:], b1_sb[:, 0:1], AF.Ln)
            nc.scalar.activation(warm[:], b1_sb[:, 0:1], AF.Exp)

            # ---------------- L1 ----------------
            gi1, da1, db1, gwt1 = (streams["gidx1"], streams["dra1"],
                                   streams["drb1"], streams["gw1"])
            from concourse.bass import _add_dep_helper as _adh

            def _add_dep_l1(a, b):
                _adh(a.ins, b.ins, sync=True, reason="p2 bank-zero after relu_b")

            def dense_l1(b, bank):
                aggT = pd.tile([128, NB1 * 128], BF16, tag="aggT", bufs=3)
                nc.scalar.activation(aggT[:], bank[:], AF.Copy)
                hstage = hstages[b % 2]
                for wl in range(NB1):
                    w = b * NB1 + wl
                    a_sl = aggT[:, wl * 128:(wl + 1) * 128]
                    # ph and p2 share one PSUM bank tile: p2's start=True
                    # zeroes the whole bank, so p2's first matmul must wait
                    # for BOTH relus (the hT dep covers relu_a; relu_b is
                    # pinned explicitly below)
                    phb = ppd.tile([128, HID + C], F32, tag="ph", bufs=3)
                    ph = phb[:, 0:HID]
                    nc.tensor.matmul(ph[:, 0:128], lhsT=W1_d[:, 0:128],
                                     rhs=a_sl, start=True, stop=False,
                                     skip_group_check=True)
                    nc.tensor.matmul(ph[:, 128:256], lhsT=W1_d[:, 128:256],
                                     rhs=a_sl, start=False, stop=True,
                                     skip_group_check=True)
                    hT = pd.tile([128, HID], BF16, tag="hT", bufs=4)
                    nc.scalar.activation(hT[:, 0:128], ph[:, 0:128],
                                         AF.Relu, bias=b1_sb[:, 0:1])
                    relu_b = nc.scalar.activation(hT[:, 128:256], ph[:, 128:256],
                                                  AF.Relu, bias=b1_sb[:, 1:2])
                    p2 = phb[:, HID:HID + C]
                    mm1 = nc.tensor.matmul(p2, lhsT=hT[:, 0:128], rhs=W2a_d[:],
                                           start=True, stop=False,
                                           skip_group_check=True)
                    _add_dep_l1(mm1, relu_b)
                    nc.tensor.matmul(p2, lhsT=hT[:, 128:256], rhs=W2b_d[:],
                                     start=False, stop=True,
                                     skip_group_check=True)
                    nc.vector.tensor_scalar(hstage[:, wl, 0:C], p2,
                                            dinv1_sb[:, w:w + 1], None, OP.mult)
                nc.sync.dma_start(
                    out=h2s_c[:, b * NB1:(b + 1) * NB1, :], in_=hstage[:])

            for b in range(NBATCH1):
                go = b * cfg.GIDX1_B
                ao = b * cfg.DRA1_B
                bank = ppa.tile([128, NB1 * 128], F32, tag="aggps")

                first = True
                for c in range(4):
                    G = pg.tile([128, NCOL1, F // 2], U32, tag="G1", bufs=6)
                    nc.gpsimd.dma_gather(
                        G[:], x[c * cfg.CH1:(c + 1) * cfg.CH1, :].bitcast(U32),
                        gi1[:, go + c * (cfg.IDX_REAL // 16):
                            go + (c + 1) * (cfg.IDX_REAL // 16)],
                        cfg.IDX_REAL, cfg.IDX_REAL, F // 2)
                    Gbf = G[:].bitcast(BF16)
                    for j, (wa, split) in enumerate(cfg.colmap):
                        for wl, second in ((wa, False), (wa + 1, True)) if split else ((wa, False),):
                            if wl >= NB1:
                                continue
                            col = ao + c * NCOL1 + j
                            S = ps.tile([128, 128], BF16, tag="S", bufs=10)
                            dsrc = db1 if second else da1
                            dcol = (b * cfg.DRB1_B + c * NCOL1 + j) if second else col
                            eng = nc.gpsimd if c == 3 else nc.vector
                            eng.tensor_scalar(
                                S[:], iota_sb[:],
                                dsrc[:, dcol:dcol + 1], gwt1[:, col:col + 1],
                                OP.is_equal, OP.mult)
                            nc.tensor.matmul(
                                bank[:, wl * 128:(wl + 1) * 128],
                                lhsT=Gbf[:, j, :], rhs=S[:],
                                start=first, stop=False, skip_group_check=True)
                            first = False
                # self
                Gs = pg.tile([128, NB1, F // 2], U32, tag="Gs1", bufs=4)
                so = go + 4 * (cfg.IDX_REAL // 16)
                nc.gpsimd.dma_gather(
                    Gs[:], x_own[:, :].bitcast(U32),
                    gi1[:, so:so + cfg.IDX_SELF // 16],
                    cfg.IDX_SELF, cfg.IDX_SELF, F // 2)
                Gsbf = Gs[:].bitcast(BF16)
                for wl in range(NB1):
                    col = ao + 4 * NCOL1 + wl
                    S = ps.tile([128, 128], BF16, tag="S", bufs=10)
                    nc.gpsimd.tensor_scalar(
                        S[:], iota_sb[:],
                        da1[:, col:col + 1], gwt1[:, col:col + 1],
                        OP.is_equal, OP.mult)
                    nc.tensor.matmul(
                        bank[:, wl * 128:(wl + 1) * 128],
                        lhsT=Gsbf[:, wl, :], rhs=S[:],
                        start=False, stop=(wl == NB1 - 1),
                        skip_group_check=True)
                dense_l1(b, bank)

            # ---------------- L2: src-sharded partials ----------------
            gi2, da2, gwt2 = streams["gidx2"], streams["dra2"], streams["gw2"]
            NB2, K2 = cfg.NB2, cfg.K2
            call_split = [w // NB2 for w in cfg.WSPLIT]

            last_gather = None
            for call in range(cfg.CALLS2):
                G2 = pg.tile([128, NB2, F // 2], U32, tag="G2", bufs=26)
                last_gather = nc.gpsimd.dma_gather(
                    G2[:],
                    h2s_c[:, :, :].rearrange("r w f -> (r w) f").bitcast(U32),
                    gi2[:, call * 64:(call + 1) * 64],
                    NB2 * K2, NB2 * K2, F // 2)
                G2bf = G2[:].bitcast(BF16)
                bank2 = ppl.tile([128, NB2 * C], F32, tag="l2ps", bufs=3)
                for wl in range(NB2):
                    w = call * NB2 + wl
                    S = ps.tile([128, 128], BF16, tag="S2", bufs=16)
                    eng = nc.vector
                    si = eng.tensor_scalar(
                        S[:], iota_sb[:],
                        da2[:, w:w + 1], gwt2[:, w:w + 1],
                        OP.is_equal, OP.mult)
                    nc.tensor.matmul(
                        bank2[:, wl * C:(wl + 1) * C],
                        lhsT=S[:], rhs=G2bf[:, wl, 0:C],
                        start=(wl == 0), stop=(wl == NB2 - 1),
                        skip_group_check=True)
                stage = pd.tile([128, NB2, C], BF16, tag="pstage", bufs=6)
                nc.scalar.activation(stage[:], bank2[:], AF.Copy)
                pj = 0 if call < call_split[0] else 1
                pcall = call - (0 if pj == 0 else call_split[0])
                last_pdma = nc.sync.dma_start(
                    out=partials[pj][:, pcall * NB2:(pcall + 1) * NB2, :],
                    in_=stage[:])

            # Collectives and the rs_out loads all live on the Pool queue,
            # explicitly chained (RS0 -> lt0 -> RS1 -> lt1) and pinned after
            # the last gather: the tile scheduler otherwise hoists the
            # collectives and stalls the L2 pipeline behind them.
            from concourse.bass import _add_dep_helper
            lts = []
            prev = last_gather
            dprev = last_pdma
            for j in range(cfg.NRS):
                cc = nc.gpsimd.collective_compute(
                    "ReduceScatter", OP.add,
                    ins=[partials[j][:, :, :]],
                    outs=[rs_out[j][:, :]],
                    replica_groups=[list(range(cfg.NC))],
                )
                _add_dep_helper(cc.ins, prev.ins, sync=True,
                                reason="keep Pool queue order")
                prev = cc
                FW = cfg.FWINS[j]
                lt = pd.tile([128, FW, C], BF16, tag=f"ltall{j}", bufs=1)
                ld = nc.sync.dma_start(
                    out=lt[:],
                    in_=rs_out[j][:, :].rearrange("(w r) c -> r w c", w=FW))
                _add_dep_helper(ld.ins, dprev.ins, sync=True,
                                reason="keep SP queue order")
                lts.append(lt)
                dprev = ld

            # ---------------- final: +b2, log_softmax on received rows ----
            # (all on DVE/ACT/SP: Pool is busy with the collectives)
            fcol = 0
            for j in range(cfg.NRS):
                FW = cfg.FWINS[j]
                lt = lts[j]
                # logits are O(+-40) so exp cannot overflow f32: skip the
                # usual max-shift and compute log_softmax = x - ln(sum(exp x)),
                # batching 8 windows per op
                xs_all = pc.tile([128, FW * C], F32, tag=f"xs{j}")
                ss_all = pc.tile([128, FW], F32, tag=f"ss{j}")
                for g0 in range(0, FW, 8):
                    gn = min(8, FW - g0)
                    xs = xs_all[:, g0 * C:(g0 + gn) * C]
                    nc.vector.tensor_add(
                        xs, lt[:, g0:g0 + gn, :], b2_sb[:, 0:gn * C])
                    es = pd.tile([128, 8 * C], F32, tag="es")
                    nc.scalar.activation(es[:, 0:gn * C], xs, AF.Exp)
                    nc.vector.tensor_reduce(
                        ss_all[:, g0:g0 + gn],
                        es[:, 0:gn * C].rearrange("p (g c) -> p g c", g=gn),
                        mybir.AxisListType.X, OP.add)
                ls_all = pc.tile([128, FW], F32, tag=f"ls{j}")
                nc.scalar.activation(ls_all[:], ss_all[:], AF.Ln)
                ost = pd.tile([128, FW, C], F32, tag=f"ost{j}", bufs=1)
                for g0 in range(0, FW, 8):
                    gn = min(8, FW - g0)
                    for fw in range(g0, g0 + gn):
                        nc.vector.tensor_scalar(
                            ost[:, fw, :], xs_all[:, fw * C:(fw + 1) * C],
                            ls_all[:, fw:fw + 1], None, OP.subtract)
                    nc.sync.dma_start(
                        out=outc[:, fcol + g0:fcol + g0 + gn, :],
                        in_=ost[:, g0:g0 + gn, :])
                fcol += FW

    nc.compile()
    return nc


_PROGRAM_CACHE = {}


def _get_program(cfg):
    key = (cfg.N, cfg.NWIN1, cfg.NB1, cfg.K1, cfg.W2, cfg.NRS)
    if key not in _PROGRAM_CACHE:
        _PROGRAM_CACHE[key] = build_program(cfg)
    return _PROGRAM_CACHE[key]


def kernel(x, W1, b1, W2, b2, edge_index):
    cfg, in_maps, owner, p, wcol = prep(x, W1, b1, W2, b2, edge_index, CFG_FULL)
    nc = _get_program(cfg)
    res = run_bass_kernel_spmd(
        nc, in_maps, core_ids=list(range(cfg.NC)),
        trace=bool(os.environ.get("GCN_TRACE")))
    if res.exec_time_ns is not None:
        print(f"HW exec time: {res.exec_time_ns} ns")
    outs = np.stack([res.results[k]["outc"] for k in range(cfg.NC)])
    return outs[owner, p, wcol].astype(np.float32)
